# revision 1
# baseline (speedup 1.0000x reference)
"""NetVLAD forward kernel for 8 TRN2 NeuronCores (Bass/Tile).

Reference (per batch b of 32):
  s = x @ Wk + b         (1024, 64) logits;  softmax over k -> a
  v[d,k] = sum_n a[n,k] x[n,d] + (sum_n a[n,k]) * C[d,k]
  v /= ||v||_2 over d (per k);  out = flatten(v) / ||flatten(v)||_2

Sharding: data-parallel over batch B=32 across 8 cores (4 batches/core).
Wk, b, C replicated; no collectives; host concatenates outputs.

Design (v5):
  - Host ships x twice in SBUF-exact layouts: xn bf16 (pixels on
    partitions, mm2 moving) + xt8 fp8e3m4 (d on partitions, mm1 moving).
    No on-chip x transposes or casts; 6MB/core HBM traffic. fp8 on the
    logits path costs ~2x bf16 rel-err (~4e-3 vs gate 2e-2); Wk stays
    bf16 (0.02-scale weights are subnormal in fp8).
  - The input wire (~19us for 6MB at ~330GB/s) is the floor; x DMAs are
    ordered xt[b+1]-before-xn[b] on one queue so mm1 never starves while
    mm2 chases the fat xn transfers.
  - Identity is built on-chip (iota + is_equal) so warmup isn't gated on
    a DMA; the transpose operand is [I64 | ones] (65 cols) so each
    e-transpose also emits the softmax row-sums Z in column 64 — no
    DVE reduction at all.
  - mm1 per n-half into s^T[64k,512n] (halves pack one PSUM bank); exp
    per half (bias=b2); 4 e-transposes/half back to a-natural; 1/Z folded
    into a with a single broadcast tensor_mul per half (DVE chain must
    stay shorter than mm1 of the next half or the PE stalls).
  - Batch emission is two-phase (all mm1/eT, then all mm2) so the PE
    runs h1's mm1 while the DVE does h0's softmax.
  - mm2: a chunks stationary, xn moving 512 wide; batch pairs pack
    v/asum PSUM rows. asum via ones-column matmuls.
  - Tail: S_k per pair by ACT Square+accum_out right after the odd
    batch; pair0's sqrt/scale/transpose/store are emitted after b3 so
    the single Exp->Sqrt table load (1.28us) and all of pair0's output
    chain hide under b3's compute; only pair1's short chain is serial.
    Global norm folded as 1/(8*sqrt(S+eps)); output stored bf16 ([d,k]
    via PE transposes), host upcasts to f32.
  - 40 warmup matmuls on the identity while DMAs land release the PE HAM
    clock gate (1.2 -> 2.4 GHz) before real work arrives.
"""

import sys

sys.path.insert(0, "/opt/trn_rl_repo")

from contextlib import ExitStack

import numpy as np

import concourse.bacc as bacc
import concourse.tile as tile
from concourse import mybir
from concourse.bass_utils import run_bass_kernel_spmd

F32 = mybir.dt.float32
BF16 = mybir.dt.bfloat16
FP8 = mybir.dt.float8e3
AX = mybir.AxisListType
ACTF = mybir.ActivationFunctionType

B_PER_CORE = 4  # 32 batches / 8 cores
N = 1024  # H*W pixels per batch
D = 512
K = 64
EPS = 1e-12
N_CORES = 8
N_WARM = 40


def build_kernel():
    nc = bacc.Bacc()
    xt8_d = nc.declare_dram_parameter("xt8", [128, 4, 2, 4, 512], FP8, isOutput=False)
    xn_d = nc.declare_dram_parameter("xn", [128, 4, 2, 4, 512], BF16, isOutput=False)
    wkb_d = nc.declare_dram_parameter("wkb", [128, 4, K], BF16, isOutput=False)
    b2_d = nc.declare_dram_parameter("b2", [128, 1], F32, isOutput=False)
    ct2_d = nc.declare_dram_parameter("ct2", [128, D], F32, isOutput=False)
    out_d = nc.declare_dram_parameter("out", [4, 128, 4, K], BF16, isOutput=True)

    with tile.TileContext(nc) as tc, ExitStack() as ctx:
        const = ctx.enter_context(tc.tile_pool(name="const", bufs=1))
        xin = ctx.enter_context(tc.tile_pool(name="xin", bufs=1))
        sb = ctx.enter_context(tc.tile_pool(name="sb", bufs=3))
        nrm = ctx.enter_context(tc.tile_pool(name="nrm", bufs=2))
        ps_s = ctx.enter_context(tc.tile_pool(name="ps_s", bufs=2, space="PSUM"))
        ps_e = ctx.enter_context(tc.tile_pool(name="ps_e", bufs=1, space="PSUM"))
        ps_v = ctx.enter_context(tc.tile_pool(name="ps_v", bufs=2, space="PSUM"))
        ps_as = ctx.enter_context(tc.tile_pool(name="ps_as", bufs=1, space="PSUM"))
        ps_o = ctx.enter_context(tc.tile_pool(name="ps_o", bufs=1, space="PSUM"))
        ps_w = ctx.enter_context(tc.tile_pool(name="ps_w", bufs=1, space="PSUM"))

        # ---- constants; identity built on-chip so warmup starts at once ----
        wkb = const.tile([128, 4, K], BF16)
        nc.scalar.dma_start(out=wkb[:], in_=wkb_d[:])
        b2 = const.tile([128, 1], F32)
        nc.scalar.dma_start(out=b2[:], in_=b2_d[:])
        ct2 = const.tile([128, D], F32)
        nc.scalar.dma_start(out=ct2[:], in_=ct2_d[:])
        ones = const.tile([128, 1], BF16)
        nc.vector.memset(ones[:], 1.0)
        eps64 = const.tile([128, 1], F32)
        nc.vector.memset(eps64[:], float(64 * EPS))
        S_all = const.tile([128, 2], F32)
        # id_aug[p, 0:64] = I64 (per 64-row half), id_aug[p, 64] = 1: the
        # transpose operand; its ones column makes each e-transpose emit
        # the row-sums Z as output column 64.
        it = const.tile([128, 65], F32)
        nc.gpsimd.iota(
            it[:], pattern=[[1, 65]], base=0, channel_multiplier=0,
            allow_small_or_imprecise_dtypes=True,
        )
        pidx = const.tile([128, 1], F32)
        nc.gpsimd.iota(
            pidx[:], pattern=[[0, 1]], base=0, channel_multiplier=1,
            allow_small_or_imprecise_dtypes=True,
        )
        pidx2 = const.tile([128, 1], F32)
        nc.gpsimd.iota(
            pidx2[:], pattern=[[0, 1]], base=-64, channel_multiplier=1,
            allow_small_or_imprecise_dtypes=True,
        )
        id_aug = const.tile([128, 65], BF16)
        nc.vector.tensor_scalar(
            id_aug[0:64, :], it[0:64, :], pidx[0:64, :], None, mybir.AluOpType.is_equal
        )
        nc.vector.tensor_scalar(
            id_aug[64:128, :], it[64:128, :], pidx2[64:128, :], None,
            mybir.AluOpType.is_equal,
        )
        nc.vector.memset(id_aug[:, 64:65], 1.0)

        # ---- x loads on one queue: xt stays a batch ahead of xn ----
        xt_all = xin.tile([128, 4, 2, 4, 512], FP8)
        xn_all = xin.tile([128, 4, 2, 4, 512], BF16)
        nc.sync.dma_start(out=xt_all[:, 0], in_=xt8_d[:, 0])
        nc.sync.dma_start(out=xt_all[:, 1], in_=xt8_d[:, 1])
        nc.sync.dma_start(out=xn_all[:, 0, 0], in_=xn_d[:, 0, 0])
        nc.sync.dma_start(out=xn_all[:, 0, 1], in_=xn_d[:, 0, 1])
        nc.sync.dma_start(out=xt_all[:, 2], in_=xt8_d[:, 2])
        nc.sync.dma_start(out=xn_all[:, 1, 0], in_=xn_d[:, 1, 0])
        nc.sync.dma_start(out=xn_all[:, 1, 1], in_=xn_d[:, 1, 1])
        nc.sync.dma_start(out=xt_all[:, 3], in_=xt8_d[:, 3])
        nc.sync.dma_start(out=xn_all[:, 2, 0], in_=xn_d[:, 2, 0])
        nc.sync.dma_start(out=xn_all[:, 2, 1], in_=xn_d[:, 2, 1])
        nc.sync.dma_start(out=xn_all[:, 3, 0], in_=xn_d[:, 3, 0])
        nc.sync.dma_start(out=xn_all[:, 3, 1], in_=xn_d[:, 3, 1])

        # ---- PE warmup: release the HAM clock gate while DMAs land ----
        warm = ps_w.tile([128, 128], F32)
        for _ in range(N_WARM):
            nc.tensor.matmul(
                warm[0:65, 0:65], id_aug[:], id_aug[:], start=True, stop=True
            )

        # ---- per-batch pipeline ----
        v2 = {}
        vvs = {}
        for b in range(B_PER_CORE):
            p2, h2 = b // 2, b % 2
            s_ps = ps_s.tile([128, 512], F32, tag="s")
            eT = sb.tile([128, 512], BF16, tag="eT")
            invz = sb.tile([128, 8], F32, tag="invz")
            a_sb = sb.tile([128, 8, K], BF16, tag="a")
            if h2 == 0:
                v_ps = ps_v.tile([128, 512], F32, tag="v")
                as_ps = ps_as.tile([128, 1], F32, tag="as")
                v2[p2] = (v_ps, as_ps)
            v_ps, as_ps = v2[p2]
            # phase 1 — mm1 + softmax per n-half (h1's mm1/eT on the PE
            # overlap h0's exp/Z/a chain on ACT+DVE)
            for h in range(2):
                e_ps = ps_e.tile([128, 4, 65], F32, tag="e")
                for j in range(4):
                    nc.tensor.matmul(
                        s_ps[64 * h : 64 * (h + 1), :],
                        wkb[:, j, :],
                        xt_all[:, b, h, j, :],
                        start=(j == 0),
                        stop=(j == 3),
                        skip_group_check=True,
                    )
                nc.scalar.activation(
                    eT[64 * h : 64 * (h + 1), :],
                    s_ps[64 * h : 64 * (h + 1), :],
                    ACTF.Exp,
                    bias=b2[64 * h : 64 * (h + 1), :],
                )
                # regular matmul (not transpose-mode): out = eT_chunk^T @
                # [I64 | 1] in f32 PSUM; column 64 = softmax row-sums Z
                for c in range(4):
                    nc.tensor.matmul(
                        e_ps[:, c, :],
                        eT[64 * h : 64 * (h + 1), c * 128 : (c + 1) * 128],
                        id_aug[64 * h : 64 * (h + 1), :],
                        start=True,
                        stop=True,
                        skip_group_check=True,
                    )
                hs = slice(4 * h, 4 * (h + 1))
                nc.vector.reciprocal(invz[:, hs], e_ps[:, :, 64])
                nc.vector.tensor_mul(
                    a_sb[:, hs, :],
                    e_ps[:, :, 0:K],
                    invz[:, hs].broadcast_to([128, 4, K]),
                )
            # phase 2 — mm2 + asum into the pair-packed PSUM rows
            for h in range(2):
                for c in range(4):
                    nc.tensor.matmul(
                        v_ps[64 * h2 : 64 * (h2 + 1), :],
                        a_sb[:, 4 * h + c, :],
                        xn_all[:, b, h, c, :],
                        start=(h == 0 and c == 0),
                        stop=(h == 1 and c == 3),
                        skip_group_check=True,
                    )
                    nc.tensor.matmul(
                        as_ps[64 * h2 : 64 * (h2 + 1), :],
                        a_sb[:, 4 * h + c, :],
                        ones[:],
                        start=(h == 0 and c == 0),
                        stop=(h == 1 and c == 3),
                        skip_group_check=True,
                    )

            if h2 == 1:
                # pair tail part 1: v = v_raw + asum*C^T; S = sum_d v^2
                vc = nrm.tile([128, D], F32, tag="vc")
                nc.vector.tensor_scalar_mul(vc[:], ct2[:], as_ps[:, 0:1])
                vv = nrm.tile([128, D], F32, tag=f"vv{p2}")
                nc.vector.tensor_add(vv[:], vc[:], v_ps[:])
                vvs[p2] = vv
                vsq = nrm.tile([128, D], F32, tag="vsq")
                nc.scalar.activation(
                    vsq[:], vv[:], ACTF.Square, accum_out=S_all[:, p2 : p2 + 1]
                )

        # ---- norm tails: pair0's chain hides under b3; pair1 is serial ----
        def finish_pair(p2):
            q8 = nrm.tile([128, 1], F32, tag="q8")
            nc.scalar.activation(
                q8[:], S_all[:, p2 : p2 + 1], ACTF.Sqrt, bias=eps64[:], scale=64.0
            )
            sc = nrm.tile([128, 1], F32, tag="sc")
            nc.vector.reciprocal(sc[:], q8[:])
            vfb = nrm.tile([128, D], BF16, tag="vfb")
            nc.vector.tensor_scalar_mul(vfb[:], vvs[p2][:], sc[:, 0:1])
            o_sb = nrm.tile([128, 2, 4, K], BF16, tag="osb")
            for hh in range(2):
                o_ps = ps_o.tile([128, 4, K], BF16, tag="o")
                for jj in range(4):
                    nc.tensor.transpose(
                        o_ps[:, jj, :],
                        vfb[64 * hh : 64 * (hh + 1), jj * 128 : (jj + 1) * 128],
                        id_aug[64 * hh : 64 * (hh + 1), 0:K],
                    )
                nc.scalar.copy(o_sb[:, hh], o_ps[:])
            nc.sync.dma_start(
                out=out_d[2 * p2 : 2 * p2 + 2].rearrange("b p j k -> p b j k"),
                in_=o_sb[:],
            )

        finish_pair(0)
        finish_pair(1)

    nc.compile()
    return nc


_CACHED_NC = None


def _get_nc():
    global _CACHED_NC
    if _CACHED_NC is None:
        _CACHED_NC = build_kernel()
    return _CACHED_NC


def build_in_maps(x, Wk, b, C):
    import ml_dtypes

    B = x.shape[0]
    x2 = np.ascontiguousarray(x, dtype=np.float32).reshape(B, N, D)
    bpc = B // N_CORES
    Wkf = np.asarray(Wk, dtype=np.float32)
    Cf = np.asarray(C, dtype=np.float32)
    bf = np.asarray(b, dtype=np.float32).reshape(K)
    consts = {
        "wkb": np.ascontiguousarray(
            Wkf.reshape(4, 128, K).transpose(1, 0, 2)
        ).astype(ml_dtypes.bfloat16),
        "ct2": np.ascontiguousarray(np.concatenate([Cf.T, Cf.T], axis=0)),
        "b2": np.concatenate([bf, bf]).reshape(128, 1),
    }
    in_maps = []
    for c in range(N_CORES):
        A = x2[c * bpc : (c + 1) * bpc]  # (4, 1024, 512)
        # xn[p, b, h, c, d]: pixel n = (4h+c)*128 + p
        xn = np.ascontiguousarray(
            A.reshape(bpc, 2, 4, 128, D).transpose(3, 0, 1, 2, 4)
        ).astype(ml_dtypes.bfloat16)
        # xt8[p, b, h, j, nn]: d = j*128 + p, n = h*512 + nn
        xt8 = np.ascontiguousarray(
            A.transpose(0, 2, 1).reshape(bpc, 4, 128, 2, 512).transpose(2, 0, 3, 1, 4)
        ).astype(ml_dtypes.float8_e3m4)
        in_maps.append({"xn": xn, "xt8": xt8, **consts})
    return in_maps


def kernel(x, Wk, b, C):
    """Full-input NetVLAD forward. x (32,32,32,512) f32 -> out (32, 32768) f32."""
    in_maps = build_in_maps(x, Wk, b, C)
    nc = _get_nc()
    res = run_bass_kernel_spmd(nc, in_maps, list(range(N_CORES)))
    outs = []
    for c in range(N_CORES):
        o = np.asarray(res.results[c]["out"])  # (4, 128, 4, 64) bf16
        outs.append(
            o.transpose(0, 2, 1, 3).reshape(B_PER_CORE, D * K).astype(np.float32)
        )
    return np.concatenate(outs, axis=0)



# revision 44
# speedup vs baseline: 1.0924x; 1.0924x over previous
"""NetVLAD forward kernel for 8 TRN2 NeuronCores (Bass/Tile).

Reference (per batch b of 32):
  s = x @ Wk + b         (1024, 64) logits;  softmax over k -> a
  v[d,k] = sum_n a[n,k] x[n,d] + (sum_n a[n,k]) * C[d,k]
  v /= ||v||_2 over d (per k);  out = flatten(v) / ||flatten(v)||_2

Sharding: data-parallel over batch B=32 across 8 cores (4 batches/core).
Wk, b, C replicated; no collectives; host concatenates outputs.

Design (v8) -- measured 34.9us best, 37-41us on slow-HBM-contention
runs, vs 44.8us baseline:
  - x ships twice in fp8 (4MB/core): xt e4m3 (d on partitions, mm1) +
    xn e3m4 (pixels on partitions, mm2). 16 half-batch descriptors on
    the sync queue, xt[b+1] ordered ahead of xn[b], so every phase pends
    on a 256KB transfer -- cushions slow HBM-contention runs. Total HW
    rel-err 1.39e-2 vs the 2e-2 gate (deterministic inputs).
  - mm1 runs fp8e4m3 DoubleRow (2 packed weights/cell, 256-deep
    contraction): 2 MULTs per n-half instead of 4, ~1.44x PE win at
    FD=512. Wk ships pre-scaled x64 (off the e4m3 subnormal floor);
    the 1/64 folds into the Exp activation scale.
  - Warmup: 8 garbage 512-col matmuls (~3.4us at the cold 1.2GHz clock,
    the HAM activity budget) so the clock gate releases ~when xt0 lands.
  - Emission interleaves phases -- P1(0) P1(1) P2(0) P1(2) P2(1) P1(3)
    T0 P2(2) P2(3) T1 -- so the in-order PE queue never head-of-line
    blocks a ready mm1 behind an xn-gated mm2.
  - P1(b): per n-half, its own [64,512] s_ps bank + eT tile at
    partitions 0:64 (no cross-half WAR -> the Tile scheduler cannot
    hoist a DMA-gated later batch ahead of the ready second half, which
    re-throttled HAM). Exp (bias=b/64-scale); 4 e-transposes against
    [I64|1] emit e-natural + row-sums Z in column 64; invz + broadcast
    mul -> a (bf16).
  - P2(b): 8 v-MULTs back-to-back (weight double-buffer stays hot),
    then 8 one-col asum MULTs (~32ns pitch); interleaving them was 2x
    slower. Batch pairs pack v/asum PSUM rows.
  - For the last batch the asum group runs BEFORE the v-MULTs so the
    pair tail is gated only by the final v-MULT, not the asum drain.
  - Tails (DVE-only + one ACT Sqrt): vv = asum*C^T + v_raw fused stt
    (bf16 out); S = sum vv^2 via stt accum_out; q = 8*sqrt(S+eps) on
    ACT -- the kernel's single Exp->Sqrt table switch,
    tile_wait_until-pinned after the model's last Exp (and pair1's
    chain pinned later still, so pair0's post-sqrt ops place first).
    vfb = vv/q in bf16.
  - Output: direct k-major stores (1-2KB runs); host does the tiny
    (64,512) per-batch transpose. THREE separate DRAM tensors (pair0,
    pair1-half0, pair1-half1) on three DMA queues (scalar / gpsimd /
    sync): Tile WAW-chains writes to a shared DRAM tensor, which had
    serialized the stores; split tensors drain in parallel, and pair1's
    first half's store overlaps its second half's scale. No PE output
    transposes, no ACT copies.
"""

import sys

sys.path.insert(0, "/opt/trn_rl_repo")

from contextlib import ExitStack

import numpy as np

import concourse.bacc as bacc
import concourse.tile as tile
from concourse import mybir
from concourse.bass_utils import run_bass_kernel_spmd

F32 = mybir.dt.float32
BF16 = mybir.dt.bfloat16
FP8 = mybir.dt.float8e3
FP8E4 = mybir.dt.float8e4
AX = mybir.AxisListType
ACTF = mybir.ActivationFunctionType

B_PER_CORE = 4  # 32 batches / 8 cores
N = 1024  # H*W pixels per batch
D = 512
K = 64
EPS = 1e-12
N_CORES = 8
N_WARM = 8


def build_kernel():
    nc = bacc.Bacc()
    xt8_d = nc.declare_dram_parameter("xt8", [128, 4, 2, 2, 2, 512], FP8E4, isOutput=False)
    xn8_d = nc.declare_dram_parameter("xn8", [128, 4, 2, 4, 512], FP8, isOutput=False)
    wkb_d = nc.declare_dram_parameter("wkb", [128, 2, 2, K], FP8E4, isOutput=False)
    b2_d = nc.declare_dram_parameter("b2", [128, 1], F32, isOutput=False)
    ct2_d = nc.declare_dram_parameter("ct2", [128, D], F32, isOutput=False)
    # three separate output tensors: Tile tracks DRAM writes per-tensor,
    # so stores to one shared tensor get WAW-chained and drain serially.
    out0_d = nc.declare_dram_parameter("out0", [128, D], BF16, isOutput=True)
    out1a_d = nc.declare_dram_parameter("out1a", [128, 256], BF16, isOutput=True)
    out1b_d = nc.declare_dram_parameter("out1b", [128, 256], BF16, isOutput=True)

    with tile.TileContext(nc) as tc, ExitStack() as ctx:
        const = ctx.enter_context(tc.tile_pool(name="const", bufs=1))
        xin = ctx.enter_context(tc.tile_pool(name="xin", bufs=1))
        sb = ctx.enter_context(tc.tile_pool(name="sb", bufs=3))
        nrm = ctx.enter_context(tc.tile_pool(name="nrm", bufs=2))
        ps_s = ctx.enter_context(tc.tile_pool(name="ps_s", bufs=3, space="PSUM"))
        ps_e = ctx.enter_context(tc.tile_pool(name="ps_e", bufs=2, space="PSUM"))
        ps_v = ctx.enter_context(tc.tile_pool(name="ps_v", bufs=2, space="PSUM"))
        ps_as = ctx.enter_context(tc.tile_pool(name="ps_as", bufs=1, space="PSUM"))

        # ---- x loads first on the sync queue: xt stays ahead of xn ----
        xt_all = xin.tile([128, 4, 2, 2, 2, 512], FP8E4)
        xn_all = xin.tile([128, 4, 2, 4, 512], FP8)
        # every x tile ships as half-batch (256KB) descriptors: each
        # mm1/mm2 phase pends on a quarter-size transfer, which cuts the
        # per-phase stall when the HBM wire ramps slowly (shared-device
        # contention); xt[b+1] halves stay ordered ahead of xn[b].
        nc.sync.dma_start(out=xt_all[:, 0, 0], in_=xt8_d[:, 0, 0])
        nc.sync.dma_start(out=xt_all[:, 0, 1], in_=xt8_d[:, 0, 1])
        nc.sync.dma_start(out=xt_all[:, 1, 0], in_=xt8_d[:, 1, 0])
        nc.sync.dma_start(out=xt_all[:, 1, 1], in_=xt8_d[:, 1, 1])
        nc.sync.dma_start(out=xn_all[:, 0, 0], in_=xn8_d[:, 0, 0])
        nc.sync.dma_start(out=xn_all[:, 0, 1], in_=xn8_d[:, 0, 1])
        nc.sync.dma_start(out=xt_all[:, 2, 0], in_=xt8_d[:, 2, 0])
        nc.sync.dma_start(out=xt_all[:, 2, 1], in_=xt8_d[:, 2, 1])
        nc.sync.dma_start(out=xn_all[:, 1, 0], in_=xn8_d[:, 1, 0])
        nc.sync.dma_start(out=xn_all[:, 1, 1], in_=xn8_d[:, 1, 1])
        nc.sync.dma_start(out=xt_all[:, 3, 0], in_=xt8_d[:, 3, 0])
        nc.sync.dma_start(out=xt_all[:, 3, 1], in_=xt8_d[:, 3, 1])
        nc.sync.dma_start(out=xn_all[:, 2, 0], in_=xn8_d[:, 2, 0])
        nc.sync.dma_start(out=xn_all[:, 2, 1], in_=xn8_d[:, 2, 1])
        nc.sync.dma_start(out=xn_all[:, 3, 0], in_=xn8_d[:, 3, 0])
        nc.sync.dma_start(out=xn_all[:, 3, 1], in_=xn8_d[:, 3, 1])

        # ---- scalar queue: tiny consts, then b0's xt halves (the
        # scalar preamble retires ~1us before sync's, so these bytes are
        # the first to land and drain in parallel with the sync queue);
        # ct2 (256KB, needed only by the pair tails) goes last ----
        wkb = const.tile([128, 2, 2, K], FP8E4)
        nc.scalar.dma_start(out=wkb[:], in_=wkb_d[:])
        b2 = const.tile([128, 1], F32)
        nc.scalar.dma_start(out=b2[:], in_=b2_d[:])
        ct2 = const.tile([128, D], F32)
        nc.scalar.dma_start(out=ct2[:], in_=ct2_d[:])

        # ---- PE warmup: ~3.4us of cold-clock matmuls releases the HAM
        # clock gate right as xt[0] lands. garb is memset first on the
        # DVE so the LDW has a ready operand without waiting on DMAs.
        garb = const.tile([128, 512], BF16)
        nc.vector.memset(garb[:], 0.25)
        # warm shares ps_v's "v" tag rotation (bank reused by pair1's v_ps)
        warm = ps_v.tile([128, 512], F32, tag="v", name="warm")
        for _ in range(N_WARM):
            nc.tensor.matmul(
                warm[:], garb[:, 0:128], garb[:], start=True, stop=True,
                skip_group_check=True,
            )

        # ---- small constants; identity built on-chip ----
        ones = const.tile([128, 1], BF16)
        nc.vector.memset(ones[:], 1.0)
        eps64 = const.tile([128, 1], F32)
        nc.vector.memset(eps64[:], float(64 * EPS))
        S_all = const.tile([128, 2], F32)
        it = const.tile([128, 65], F32)
        nc.gpsimd.iota(
            it[:], pattern=[[1, 65]], base=0, channel_multiplier=0,
            allow_small_or_imprecise_dtypes=True,
        )
        pidx = const.tile([128, 1], F32)
        nc.gpsimd.iota(
            pidx[:], pattern=[[0, 1]], base=0, channel_multiplier=1,
            allow_small_or_imprecise_dtypes=True,
        )
        pidx2 = const.tile([128, 1], F32)
        nc.gpsimd.iota(
            pidx2[:], pattern=[[0, 1]], base=-64, channel_multiplier=1,
            allow_small_or_imprecise_dtypes=True,
        )
        id_aug = const.tile([128, 65], BF16)
        nc.vector.tensor_scalar(
            id_aug[0:64, :], it[0:64, :], pidx[0:64, :], None, mybir.AluOpType.is_equal
        )
        nc.vector.tensor_scalar(
            id_aug[64:128, :], it[64:128, :], pidx2[64:128, :], None,
            mybir.AluOpType.is_equal,
        )
        nc.vector.memset(id_aug[:, 64:65], 1.0)

        v2 = {}
        as2 = {}

        def phase1(b):
            """mm1 + softmax -> a_sb[b]  (PE: 8x512 MULTs + 8 e-transposes).
            Each n-half gets its own [64,512] s_ps bank and eT tile at
            partitions 0:64 -- no cross-half WAR on a shared tile, so the
            scheduler never hoists a DMA-gated later batch ahead of the
            ready second half (which stalled the PE into a HAM re-throttle)."""
            invz = sb.tile([128, 8], F32, tag="invz")
            a_sb = sb.tile([128, 8, K], BF16, tag=f"a{b}")
            for h in range(2):
                s_ps = ps_s.tile([64, 512], F32, tag="s", name=f"s{b}h{h}", bufs=3)
                eT = sb.tile([64, 512], BF16, tag="eT", name=f"eT{b}h{h}", bufs=3)
                e_ps = ps_e.tile([128, 4, 65], F32, tag="e", bufs=2)
                # DoubleRow fp8: 2 packed weights/cell, 256-deep
                # contraction per MULT -- 2 MULTs per half instead of 4.
                # Wk ships pre-scaled by 64 (e4m3 subnormal floor); the
                # 1/64 folds into the Exp scale.
                for j in range(2):
                    nc.tensor.matmul(
                        s_ps[:],
                        wkb[:, j],
                        xt_all[:, b, h, j],
                        start=(j == 0),
                        stop=(j == 1),
                        perf_mode=mybir.MatmulPerfMode.DoubleRow,
                        skip_group_check=True,
                    )
                nc.scalar.activation(
                    eT[:], s_ps[:], ACTF.Exp, bias=b2[0:64, :], scale=0.015625
                )
                # out = eT_chunk^T @ [I64 | 1]: e natural + row-sums Z in col 64
                for c in range(4):
                    nc.tensor.matmul(
                        e_ps[:, c, :],
                        eT[:, c * 128 : (c + 1) * 128],
                        id_aug[0:64, :],
                        start=True,
                        stop=True,
                        skip_group_check=True,
                    )
                hs = slice(4 * h, 4 * (h + 1))
                nc.vector.reciprocal(invz[:, hs], e_ps[:, :, 64])
                nc.vector.tensor_mul(
                    a_sb[:, hs, :],
                    e_ps[:, :, 0:K],
                    invz[:, hs].broadcast_to([128, 4, K]),
                )
            return a_sb

        def phase2(b, a_sb):
            """mm2 + asum into pair-packed PSUM rows. For the last batch the
            asum group runs FIRST so the pair tail's vv (which needs both
            v_ps and as_ps) is gated only by the final v-MULT."""
            p2, h2 = b // 2, b % 2
            if h2 == 0:
                v2[p2] = ps_v.tile([128, 512], F32, tag="v", name=f"vps{p2}")
                as2[p2] = ps_as.tile([128, 1], F32, tag="as", name=f"as{p2}")
            v_ps = v2[p2]
            as_ps = as2[p2]
            rows = slice(64 * h2, 64 * (h2 + 1))

            def v_group():
                # all 8 v-MULTs back-to-back: weight double-buffer stays hot
                for c8 in range(8):
                    nc.tensor.matmul(
                        v_ps[rows, :],
                        a_sb[:, c8, :],
                        xn_all[:, b, c8 // 4, c8 % 4, :],
                        start=(c8 == 0),
                        stop=(c8 == 7),
                        skip_group_check=True,
                    )

            def as_group():
                # grouped 1-col asum MULTs (re-LDW is cheap; interleaving isn't)
                for c8 in range(8):
                    nc.tensor.matmul(
                        as_ps[rows, :],
                        a_sb[:, c8, :],
                        ones[:],
                        start=(c8 == 0),
                        stop=(c8 == 7),
                        skip_group_check=True,
                    )

            if b == 3:
                as_group()
                v_group()
            else:
                v_group()
                as_group()

        def tail_norm(p2):
            """v = asum*C^T + v_raw (fused stt); S = sum_d v^2 via accum_out;
            q = 8*sqrt(S+eps) on ACT (single Exp->Sqrt table switch);
            vfb = v/q. Pins: only SQRT(p0) (just after the model's last
            Exp, so the Sqrt table loads once) and pair1's whole chain
            (later still, so pair0's post-sqrt ops place first on the
            DVE and its store issues early)."""
            v_ps = v2[p2]
            A = mybir.AluOpType
            # bf16 intermediates: 2x DVE rate on the serial pair1 chain;
            # S still accumulates in f32 via accum_out
            vv = nrm.tile([128, D], BF16, tag=f"vv{p2}", name=f"vv{p2}")
            vsq = nrm.tile([128, D], BF16, tag=f"vsq{p2}", name=f"vsq{p2}")
            q = nrm.tile([128, 1], F32, tag=f"q{p2}", name=f"q{p2}")
            sc = nrm.tile([128, 1], F32, tag=f"sc{p2}", name=f"sc{p2}")
            vfb = nrm.tile([128, D], BF16, tag=f"vfb{p2}", name=f"vfb{p2}")
            with tc.tile_wait_until(0.028, enable=(p2 == 1)):
                nc.vector.scalar_tensor_tensor(
                    vv[:], ct2[:], as2[p2][:, 0:1], v_ps[:], A.mult, A.add
                )
                nc.vector.scalar_tensor_tensor(
                    vsq[:], vv[:], 1.0, vv[:], A.bypass, A.mult,
                    accum_out=S_all[:, p2 : p2 + 1],
                )
                with tc.tile_wait_until(0.024, enable=(p2 == 0)):
                    nc.scalar.activation(
                        q[:], S_all[:, p2 : p2 + 1], ACTF.Sqrt,
                        bias=eps64[:], scale=64.0,
                    )
                nc.vector.reciprocal(sc[:], q[:])
                if p2 == 1:
                    # split the final scale+store into column halves; each
                    # half has its own DRAM tensor and DMA queue (gpsimd /
                    # sync -- both idle) so the two 64KB stores drain in
                    # parallel and overlap the second half's scale
                    nc.vector.tensor_scalar_mul(vfb[:, 0:256], vv[:, 0:256], sc[:, 0:1])
                    nc.gpsimd.dma_start(out=out1a_d[:], in_=vfb[:, 0:256])
                    nc.vector.tensor_scalar_mul(vfb[:, 256:512], vv[:, 256:512], sc[:, 0:1])
                    nc.sync.dma_start(out=out1b_d[:], in_=vfb[:, 256:512])
                else:
                    nc.vector.tensor_scalar_mul(vfb[:], vv[:], sc[:, 0:1])
            return vfb

        def tail_store(p2, vfb):
            """Direct k-major store (1KB runs); host does the tiny (64,512)
            per-batch transpose. Pair0 goes out on the scalar engine's DMA
            queue so it never waits behind the x input stream; pair1
            stores inline in tail_norm (split halves, two queues)."""
            nc.scalar.dma_start(out=out0_d[:], in_=vfb[:])

        # ---- interleaved schedule: no PE head-of-line blocking ----
        a0 = phase1(0)
        a1 = phase1(1)
        phase2(0, a0)
        a2 = phase1(2)
        phase2(1, a1)
        a3 = phase1(3)
        vfb0 = tail_norm(0)  # DVE-only; fills DVE idle gaps
        tail_store(0, vfb0)
        phase2(2, a2)
        phase2(3, a3)
        tail_norm(1)  # pair1 stores inside (split halves)

    nc.compile()
    return nc


_CACHED_NC = None


def _get_nc():
    global _CACHED_NC
    if _CACHED_NC is None:
        _CACHED_NC = build_kernel()
    return _CACHED_NC


def build_in_maps(x, Wk, b, C):
    import ml_dtypes

    B = x.shape[0]
    x2 = np.ascontiguousarray(x, dtype=np.float32).reshape(B, N, D)
    bpc = B // N_CORES
    Wkf = np.asarray(Wk, dtype=np.float32)
    Cf = np.asarray(C, dtype=np.float32)
    bf = np.asarray(b, dtype=np.float32).reshape(K)
    consts = {
        # d = j*256 + o*128 + p; DoubleRow packs (o) pairs per PE cell.
        # x64 lifts the 0.02-scale weights off the e4m3 subnormal floor.
        "wkb": np.ascontiguousarray(
            (Wkf * 64.0).reshape(2, 2, 128, K).transpose(2, 0, 1, 3)
        ).astype(ml_dtypes.float8_e4m3),
        "ct2": np.ascontiguousarray(np.concatenate([Cf.T, Cf.T], axis=0)),
        "b2": np.concatenate([bf, bf]).reshape(128, 1),
    }
    in_maps = []
    for c in range(N_CORES):
        A = x2[c * bpc : (c + 1) * bpc]  # (4, 1024, 512)
        # xn8[p, b, h, c, d]: pixel n = (4h+c)*128 + p
        xn8 = np.ascontiguousarray(
            A.reshape(bpc, 2, 4, 128, D).transpose(3, 0, 1, 2, 4)
        ).astype(ml_dtypes.float8_e3m4)
        # xt8[p, b, h, j, o, nn]: d = j*256 + o*128 + p, n = h*512 + nn
        xt8 = np.ascontiguousarray(
            A.transpose(0, 2, 1)
            .reshape(bpc, 2, 2, 128, 2, 512)
            .transpose(3, 0, 4, 1, 2, 5)
        ).astype(ml_dtypes.float8_e4m3)
        in_maps.append({"xn8": xn8, "xt8": xt8, **consts})
    return in_maps


def kernel(x, Wk, b, C):
    """Full-input NetVLAD forward. x (32,32,32,512) f32 -> out (32, 32768) f32."""
    in_maps = build_in_maps(x, Wk, b, C)
    nc = _get_nc()
    res = run_bass_kernel_spmd(nc, in_maps, list(range(N_CORES)))
    outs = []
    for c in range(N_CORES):
        o = np.stack(
            [
                np.asarray(res.results[c]["out0"]),
                np.concatenate(
                    [
                        np.asarray(res.results[c]["out1a"]),
                        np.asarray(res.results[c]["out1b"]),
                    ],
                    axis=1,
                ),
            ],
            axis=1,
        )  # (128, 2, 512) bf16: p=64*hh+k
        outs.append(
            o.reshape(2, K, 2, D)
            .transpose(2, 0, 3, 1)
            .reshape(B_PER_CORE, D * K)
            .astype(np.float32)
        )
    return np.concatenate(outs, axis=0)


# revision 45
# speedup vs baseline: 1.1534x; 1.0559x over previous
"""NetVLAD forward kernel for 8 TRN2 NeuronCores (Bass/Tile).

Reference (per batch b of 32):
  s = x @ Wk + b         (1024, 64) logits;  softmax over k -> a
  v[d,k] = sum_n a[n,k] x[n,d] + (sum_n a[n,k]) * C[d,k]
  v /= ||v||_2 over d (per k);  out = flatten(v) / ||flatten(v)||_2

Sharding: data-parallel over batch B=32 across 8 cores (4 batches/core).
Wk, b, C replicated; no collectives; host concatenates outputs.

Design (v8) -- measured 34.9us best, 37-41us on slow-HBM-contention
runs, vs 44.8us baseline:
  - x ships twice in fp8 (4MB/core): xt e4m3 (d on partitions, mm1) +
    xn e3m4 (pixels on partitions, mm2). 16 half-batch descriptors on
    the sync queue, xt[b+1] ordered ahead of xn[b], so every phase pends
    on a 256KB transfer -- cushions slow HBM-contention runs. Total HW
    rel-err 1.39e-2 vs the 2e-2 gate (deterministic inputs).
  - mm1 runs fp8e4m3 DoubleRow (2 packed weights/cell, 256-deep
    contraction): 2 MULTs per n-half instead of 4, ~1.44x PE win at
    FD=512. Wk ships pre-scaled x64 (off the e4m3 subnormal floor);
    the 1/64 folds into the Exp activation scale.
  - Warmup: 8 garbage 512-col matmuls (~3.4us at the cold 1.2GHz clock,
    the HAM activity budget) so the clock gate releases ~when xt0 lands.
  - Emission interleaves phases -- P1(0) P1(1) P2(0) P1(2) P2(1) P1(3)
    T0 P2(2) P2(3) T1 -- so the in-order PE queue never head-of-line
    blocks a ready mm1 behind an xn-gated mm2.
  - P1(b): per n-half, its own [64,512] s_ps bank + eT tile at
    partitions 0:64 (no cross-half WAR -> the Tile scheduler cannot
    hoist a DMA-gated later batch ahead of the ready second half, which
    re-throttled HAM). Exp (bias=b/64-scale); 4 e-transposes against
    [I64|1] emit e-natural + row-sums Z in column 64; invz + broadcast
    mul -> a (bf16).
  - P2(b): 8 v-MULTs back-to-back (weight double-buffer stays hot),
    then 8 one-col asum MULTs (~32ns pitch); interleaving them was 2x
    slower. Batch pairs pack v/asum PSUM rows.
  - For the last batch the asum group runs BEFORE the v-MULTs so the
    pair tail is gated only by the final v-MULT, not the asum drain.
  - Tails (DVE-only + one ACT Sqrt): vv = asum*C^T + v_raw fused stt
    (bf16 out); S = sum vv^2 via stt accum_out; q = 8*sqrt(S+eps) on
    ACT -- the kernel's single Exp->Sqrt table switch,
    tile_wait_until-pinned after the model's last Exp (and pair1's
    chain pinned later still, so pair0's post-sqrt ops place first).
    vfb = vv/q in bf16.
  - Output: direct k-major stores (1-2KB runs); host does the tiny
    (64,512) per-batch transpose. THREE separate DRAM tensors (pair0,
    pair1-half0, pair1-half1) on three DMA queues (scalar / gpsimd /
    sync): Tile WAW-chains writes to a shared DRAM tensor, which had
    serialized the stores; split tensors drain in parallel, and pair1's
    first half's store overlaps its second half's scale. No PE output
    transposes, no ACT copies.
"""

import sys

sys.path.insert(0, "/opt/trn_rl_repo")

from contextlib import ExitStack

import numpy as np

import concourse.bacc as bacc
import concourse.tile as tile
from concourse import mybir
from concourse.bass_utils import run_bass_kernel_spmd

F32 = mybir.dt.float32
BF16 = mybir.dt.bfloat16
FP8 = mybir.dt.float8e3
FP8E4 = mybir.dt.float8e4
AX = mybir.AxisListType
ACTF = mybir.ActivationFunctionType

B_PER_CORE = 4  # 32 batches / 8 cores
N = 1024  # H*W pixels per batch
D = 512
K = 64
EPS = 1e-12
N_CORES = 8
N_WARM = 8


def build_kernel():
    nc = bacc.Bacc()
    xt8_d = nc.declare_dram_parameter("xt8", [128, 4, 2, 2, 2, 512], FP8E4, isOutput=False)
    xn8_d = nc.declare_dram_parameter("xn8", [128, 4, 2, 4, 512], FP8, isOutput=False)
    wkb_d = nc.declare_dram_parameter("wkb", [128, 2, 2, K], FP8E4, isOutput=False)
    b2_d = nc.declare_dram_parameter("b2", [128, 1], F32, isOutput=False)
    ct2_d = nc.declare_dram_parameter("ct2", [128, D], F32, isOutput=False)
    # three separate output tensors: Tile tracks DRAM writes per-tensor,
    # so stores to one shared tensor get WAW-chained and drain serially.
    out0_d = nc.declare_dram_parameter("out0", [128, D], BF16, isOutput=True)
    out1a_d = nc.declare_dram_parameter("out1a", [128, 256], BF16, isOutput=True)
    out1b_d = nc.declare_dram_parameter("out1b", [128, 256], BF16, isOutput=True)

    with tile.TileContext(nc) as tc, ExitStack() as ctx:
        const = ctx.enter_context(tc.tile_pool(name="const", bufs=1))
        xin = ctx.enter_context(tc.tile_pool(name="xin", bufs=1))
        sb = ctx.enter_context(tc.tile_pool(name="sb", bufs=3))
        nrm = ctx.enter_context(tc.tile_pool(name="nrm", bufs=2))
        ps_s = ctx.enter_context(tc.tile_pool(name="ps_s", bufs=3, space="PSUM"))
        ps_e = ctx.enter_context(tc.tile_pool(name="ps_e", bufs=2, space="PSUM"))
        ps_v = ctx.enter_context(tc.tile_pool(name="ps_v", bufs=2, space="PSUM"))
        ps_as = ctx.enter_context(tc.tile_pool(name="ps_as", bufs=1, space="PSUM"))

        # ---- x loads first on the sync queue: xt stays ahead of xn ----
        xt_all = xin.tile([128, 4, 2, 2, 2, 512], FP8E4)
        xn_all = xin.tile([128, 4, 2, 4, 512], FP8)
        # every x tile ships as half-batch (256KB) descriptors: each
        # mm1/mm2 phase pends on a quarter-size transfer, which cuts the
        # per-phase stall when the HBM wire ramps slowly (shared-device
        # contention); xt[b+1] halves stay ordered ahead of xn[b].
        nc.sync.dma_start(out=xt_all[:, 0, 0], in_=xt8_d[:, 0, 0])
        nc.sync.dma_start(out=xt_all[:, 0, 1], in_=xt8_d[:, 0, 1])
        nc.sync.dma_start(out=xt_all[:, 1, 0], in_=xt8_d[:, 1, 0])
        nc.sync.dma_start(out=xt_all[:, 1, 1], in_=xt8_d[:, 1, 1])
        nc.sync.dma_start(out=xn_all[:, 0, 0], in_=xn8_d[:, 0, 0])
        nc.sync.dma_start(out=xn_all[:, 0, 1], in_=xn8_d[:, 0, 1])
        nc.sync.dma_start(out=xt_all[:, 2, 0], in_=xt8_d[:, 2, 0])
        nc.sync.dma_start(out=xt_all[:, 2, 1], in_=xt8_d[:, 2, 1])
        nc.sync.dma_start(out=xn_all[:, 1, 0], in_=xn8_d[:, 1, 0])
        nc.sync.dma_start(out=xn_all[:, 1, 1], in_=xn8_d[:, 1, 1])
        nc.sync.dma_start(out=xt_all[:, 3, 0], in_=xt8_d[:, 3, 0])
        nc.sync.dma_start(out=xt_all[:, 3, 1], in_=xt8_d[:, 3, 1])
        nc.sync.dma_start(out=xn_all[:, 2, 0], in_=xn8_d[:, 2, 0])
        nc.sync.dma_start(out=xn_all[:, 2, 1], in_=xn8_d[:, 2, 1])
        # xn3 ships as quarter-batch (128KB) descriptors: the tail waits
        # on the LAST arrival, and halving the final transfer halves the
        # v-MULT work still pending when the wire finishes
        nc.sync.dma_start(out=xn_all[:, 3, 0, 0:2], in_=xn8_d[:, 3, 0, 0:2])
        nc.sync.dma_start(out=xn_all[:, 3, 0, 2:4], in_=xn8_d[:, 3, 0, 2:4])
        nc.sync.dma_start(out=xn_all[:, 3, 1, 0:2], in_=xn8_d[:, 3, 1, 0:2])
        nc.sync.dma_start(out=xn_all[:, 3, 1, 2:4], in_=xn8_d[:, 3, 1, 2:4])

        # ---- scalar queue: tiny consts, then b0's xt halves (the
        # scalar preamble retires ~1us before sync's, so these bytes are
        # the first to land and drain in parallel with the sync queue);
        # ct2 (256KB, needed only by the pair tails) goes last ----
        wkb = const.tile([128, 2, 2, K], FP8E4)
        nc.scalar.dma_start(out=wkb[:], in_=wkb_d[:])
        b2 = const.tile([128, 1], F32)
        nc.scalar.dma_start(out=b2[:], in_=b2_d[:])
        ct2 = const.tile([128, D], F32)
        nc.scalar.dma_start(out=ct2[:], in_=ct2_d[:])

        # ---- PE warmup: ~3.4us of cold-clock matmuls releases the HAM
        # clock gate right as xt[0] lands. garb is memset first on the
        # DVE so the LDW has a ready operand without waiting on DMAs.
        garb = const.tile([128, 512], BF16)
        nc.vector.memset(garb[:], 0.25)
        # warm shares ps_v's "v" tag rotation (bank reused by pair1's v_ps)
        warm = ps_v.tile([128, 512], F32, tag="v", name="warm")
        for _ in range(N_WARM):
            nc.tensor.matmul(
                warm[:], garb[:, 0:128], garb[:], start=True, stop=True,
                skip_group_check=True,
            )

        # ---- small constants; identity built on-chip ----
        ones = const.tile([128, 1], BF16)
        nc.vector.memset(ones[:], 1.0)
        eps64 = const.tile([128, 1], F32)
        nc.vector.memset(eps64[:], float(64 * EPS))
        S_all = const.tile([128, 2], F32)
        it = const.tile([128, 65], F32)
        nc.gpsimd.iota(
            it[:], pattern=[[1, 65]], base=0, channel_multiplier=0,
            allow_small_or_imprecise_dtypes=True,
        )
        pidx = const.tile([128, 1], F32)
        nc.gpsimd.iota(
            pidx[:], pattern=[[0, 1]], base=0, channel_multiplier=1,
            allow_small_or_imprecise_dtypes=True,
        )
        pidx2 = const.tile([128, 1], F32)
        nc.gpsimd.iota(
            pidx2[:], pattern=[[0, 1]], base=-64, channel_multiplier=1,
            allow_small_or_imprecise_dtypes=True,
        )
        id_aug = const.tile([128, 65], BF16)
        nc.vector.tensor_scalar(
            id_aug[0:64, :], it[0:64, :], pidx[0:64, :], None, mybir.AluOpType.is_equal
        )
        nc.vector.tensor_scalar(
            id_aug[64:128, :], it[64:128, :], pidx2[64:128, :], None,
            mybir.AluOpType.is_equal,
        )
        nc.vector.memset(id_aug[:, 64:65], 1.0)

        v2 = {}
        as2 = {}

        def phase1(b):
            """mm1 + softmax -> a_sb[b]  (PE: 8x512 MULTs + 8 e-transposes).
            Each n-half gets its own [64,512] s_ps bank and eT tile at
            partitions 0:64 -- no cross-half WAR on a shared tile, so the
            scheduler never hoists a DMA-gated later batch ahead of the
            ready second half (which stalled the PE into a HAM re-throttle)."""
            invz = sb.tile([128, 8], F32, tag="invz")
            a_sb = sb.tile([128, 8, K], BF16, tag=f"a{b}")
            for h in range(2):
                s_ps = ps_s.tile([64, 512], F32, tag="s", name=f"s{b}h{h}", bufs=3)
                eT = sb.tile([64, 512], BF16, tag="eT", name=f"eT{b}h{h}", bufs=3)
                e_ps = ps_e.tile([128, 4, 65], F32, tag="e", bufs=2)
                # DoubleRow fp8: 2 packed weights/cell, 256-deep
                # contraction per MULT -- 2 MULTs per half instead of 4.
                # Wk ships pre-scaled by 64 (e4m3 subnormal floor); the
                # 1/64 folds into the Exp scale.
                for j in range(2):
                    nc.tensor.matmul(
                        s_ps[:],
                        wkb[:, j],
                        xt_all[:, b, h, j],
                        start=(j == 0),
                        stop=(j == 1),
                        perf_mode=mybir.MatmulPerfMode.DoubleRow,
                        skip_group_check=True,
                    )
                nc.scalar.activation(
                    eT[:], s_ps[:], ACTF.Exp, bias=b2[0:64, :], scale=0.015625
                )
                # out = eT_chunk^T @ [I64 | 1]: e natural + row-sums Z in col 64
                for c in range(4):
                    nc.tensor.matmul(
                        e_ps[:, c, :],
                        eT[:, c * 128 : (c + 1) * 128],
                        id_aug[0:64, :],
                        start=True,
                        stop=True,
                        skip_group_check=True,
                    )
                hs = slice(4 * h, 4 * (h + 1))
                nc.vector.reciprocal(invz[:, hs], e_ps[:, :, 64])
                nc.vector.tensor_mul(
                    a_sb[:, hs, :],
                    e_ps[:, :, 0:K],
                    invz[:, hs].broadcast_to([128, 4, K]),
                )
            return a_sb

        def phase2(b, a_sb):
            """mm2 + asum into pair-packed PSUM rows. For the last batch the
            asum group runs FIRST so the pair tail's vv (which needs both
            v_ps and as_ps) is gated only by the final v-MULT."""
            p2, h2 = b // 2, b % 2
            if h2 == 0:
                v2[p2] = ps_v.tile([128, 512], F32, tag="v", name=f"vps{p2}")
                as2[p2] = ps_as.tile([128, 1], F32, tag="as", name=f"as{p2}")
            v_ps = v2[p2]
            as_ps = as2[p2]
            rows = slice(64 * h2, 64 * (h2 + 1))

            def v_group():
                # all 8 v-MULTs back-to-back: weight double-buffer stays hot
                for c8 in range(8):
                    nc.tensor.matmul(
                        v_ps[rows, :],
                        a_sb[:, c8, :],
                        xn_all[:, b, c8 // 4, c8 % 4, :],
                        start=(c8 == 0),
                        stop=(c8 == 7),
                        skip_group_check=True,
                    )

            def as_group():
                # grouped 1-col asum MULTs (re-LDW is cheap; interleaving isn't)
                for c8 in range(8):
                    nc.tensor.matmul(
                        as_ps[rows, :],
                        a_sb[:, c8, :],
                        ones[:],
                        start=(c8 == 0),
                        stop=(c8 == 7),
                        skip_group_check=True,
                    )

            if b == 3:
                as_group()
                v_group()
            else:
                v_group()
                as_group()

        def tail_norm(p2):
            """v = asum*C^T + v_raw (fused stt); S = sum_d v^2 via accum_out;
            q = 8*sqrt(S+eps) on ACT (single Exp->Sqrt table switch);
            vfb = v/q. Pins: only SQRT(p0) (just after the model's last
            Exp, so the Sqrt table loads once) and pair1's whole chain
            (later still, so pair0's post-sqrt ops place first on the
            DVE and its store issues early)."""
            v_ps = v2[p2]
            A = mybir.AluOpType
            # bf16 intermediates: 2x DVE rate on the serial pair1 chain;
            # S still accumulates in f32 via accum_out
            vv = nrm.tile([128, D], BF16, tag=f"vv{p2}", name=f"vv{p2}")
            vsq = nrm.tile([128, D], BF16, tag=f"vsq{p2}", name=f"vsq{p2}")
            q = nrm.tile([128, 1], F32, tag=f"q{p2}", name=f"q{p2}")
            sc = nrm.tile([128, 1], F32, tag=f"sc{p2}", name=f"sc{p2}")
            vfb = nrm.tile([128, D], BF16, tag=f"vfb{p2}", name=f"vfb{p2}")
            with tc.tile_wait_until(0.028, enable=(p2 == 1)):
                nc.vector.scalar_tensor_tensor(
                    vv[:], ct2[:], as2[p2][:, 0:1], v_ps[:], A.mult, A.add
                )
                nc.vector.scalar_tensor_tensor(
                    vsq[:], vv[:], 1.0, vv[:], A.bypass, A.mult,
                    accum_out=S_all[:, p2 : p2 + 1],
                )
                with tc.tile_wait_until(0.024, enable=(p2 == 0)):
                    nc.scalar.activation(
                        q[:], S_all[:, p2 : p2 + 1], ACTF.Sqrt,
                        bias=eps64[:], scale=64.0,
                    )
                nc.vector.reciprocal(sc[:], q[:])
                if p2 == 1:
                    # split the final scale+store into column halves; each
                    # half has its own DRAM tensor and DMA queue (gpsimd /
                    # sync -- both idle) so the two 64KB stores drain in
                    # parallel and overlap the second half's scale
                    nc.vector.tensor_scalar_mul(vfb[:, 0:256], vv[:, 0:256], sc[:, 0:1])
                    nc.gpsimd.dma_start(out=out1a_d[:], in_=vfb[:, 0:256])
                    nc.vector.tensor_scalar_mul(vfb[:, 256:512], vv[:, 256:512], sc[:, 0:1])
                    nc.sync.dma_start(out=out1b_d[:], in_=vfb[:, 256:512])
                else:
                    nc.vector.tensor_scalar_mul(vfb[:], vv[:], sc[:, 0:1])
            return vfb

        def tail_store(p2, vfb):
            """Direct k-major store (1KB runs); host does the tiny (64,512)
            per-batch transpose. Pair0 goes out on the scalar engine's DMA
            queue so it never waits behind the x input stream; pair1
            stores inline in tail_norm (split halves, two queues)."""
            nc.scalar.dma_start(out=out0_d[:], in_=vfb[:])

        # ---- interleaved schedule: no PE head-of-line blocking ----
        a0 = phase1(0)
        a1 = phase1(1)
        phase2(0, a0)
        a2 = phase1(2)
        phase2(1, a1)
        a3 = phase1(3)
        vfb0 = tail_norm(0)  # DVE-only; fills DVE idle gaps
        tail_store(0, vfb0)
        phase2(2, a2)
        phase2(3, a3)
        tail_norm(1)  # pair1 stores inside (split halves)

    nc.compile()
    return nc


_CACHED_NC = None


def _get_nc():
    global _CACHED_NC
    if _CACHED_NC is None:
        _CACHED_NC = build_kernel()
    return _CACHED_NC


def build_in_maps(x, Wk, b, C):
    import ml_dtypes

    B = x.shape[0]
    x2 = np.ascontiguousarray(x, dtype=np.float32).reshape(B, N, D)
    bpc = B // N_CORES
    Wkf = np.asarray(Wk, dtype=np.float32)
    Cf = np.asarray(C, dtype=np.float32)
    bf = np.asarray(b, dtype=np.float32).reshape(K)
    consts = {
        # d = j*256 + o*128 + p; DoubleRow packs (o) pairs per PE cell.
        # x64 lifts the 0.02-scale weights off the e4m3 subnormal floor.
        "wkb": np.ascontiguousarray(
            (Wkf * 64.0).reshape(2, 2, 128, K).transpose(2, 0, 1, 3)
        ).astype(ml_dtypes.float8_e4m3),
        "ct2": np.ascontiguousarray(np.concatenate([Cf.T, Cf.T], axis=0)),
        "b2": np.concatenate([bf, bf]).reshape(128, 1),
    }
    in_maps = []
    for c in range(N_CORES):
        A = x2[c * bpc : (c + 1) * bpc]  # (4, 1024, 512)
        # xn8[p, b, h, c, d]: pixel n = (4h+c)*128 + p
        xn8 = np.ascontiguousarray(
            A.reshape(bpc, 2, 4, 128, D).transpose(3, 0, 1, 2, 4)
        ).astype(ml_dtypes.float8_e3m4)
        # xt8[p, b, h, j, o, nn]: d = j*256 + o*128 + p, n = h*512 + nn
        xt8 = np.ascontiguousarray(
            A.transpose(0, 2, 1)
            .reshape(bpc, 2, 2, 128, 2, 512)
            .transpose(3, 0, 4, 1, 2, 5)
        ).astype(ml_dtypes.float8_e4m3)
        in_maps.append({"xn8": xn8, "xt8": xt8, **consts})
    return in_maps


def kernel(x, Wk, b, C):
    """Full-input NetVLAD forward. x (32,32,32,512) f32 -> out (32, 32768) f32."""
    in_maps = build_in_maps(x, Wk, b, C)
    nc = _get_nc()
    res = run_bass_kernel_spmd(nc, in_maps, list(range(N_CORES)))
    outs = []
    for c in range(N_CORES):
        o = np.stack(
            [
                np.asarray(res.results[c]["out0"]),
                np.concatenate(
                    [
                        np.asarray(res.results[c]["out1a"]),
                        np.asarray(res.results[c]["out1b"]),
                    ],
                    axis=1,
                ),
            ],
            axis=1,
        )  # (128, 2, 512) bf16: p=64*hh+k
        outs.append(
            o.reshape(2, K, 2, D)
            .transpose(2, 0, 3, 1)
            .reshape(B_PER_CORE, D * K)
            .astype(np.float32)
        )
    return np.concatenate(outs, axis=0)


# revision 47
# speedup vs baseline: 1.2377x; 1.0731x over previous
"""NetVLAD forward kernel for 8 TRN2 NeuronCores (Bass/Tile).

Reference (per batch b of 32):
  s = x @ Wk + b         (1024, 64) logits;  softmax over k -> a
  v[d,k] = sum_n a[n,k] x[n,d] + (sum_n a[n,k]) * C[d,k]
  v /= ||v||_2 over d (per k);  out = flatten(v) / ||flatten(v)||_2

Sharding: data-parallel over batch B=32 across 8 cores (4 batches/core).
Wk, b, C replicated; no collectives; host concatenates outputs.

Design (v8) -- measured 34.9us best, 37-41us on slow-HBM-contention
runs, vs 44.8us baseline:
  - x ships twice in fp8 (4MB/core): xt e4m3 (d on partitions, mm1) +
    xn e3m4 (pixels on partitions, mm2). Half-batch (256KB) descriptors
    on the sync queue, xt[b+1] ordered ahead of xn[b], so every phase
    pends on a small transfer -- cushions slow HBM-contention runs; the
    last tile (xn3) ships as 128KB quarters so the tail's final v-MULTs
    pend on the smallest possible arrival. Total HW rel-err 1.39e-2 vs
    the 2e-2 gate (deterministic inputs).
  - mm1 runs fp8e4m3 DoubleRow (2 packed weights/cell, 256-deep
    contraction): 2 MULTs per n-half instead of 4, ~1.44x PE win at
    FD=512. Wk ships pre-scaled x64 (off the e4m3 subnormal floor);
    the 1/64 folds into the Exp activation scale.
  - Warmup: 8 garbage 512-col matmuls (~3.4us at the cold 1.2GHz clock,
    the HAM activity budget) so the clock gate releases ~when xt0 lands.
  - Emission interleaves phases -- P1(0) P1(1) P2(0) P1(2) P2(1) P1(3)
    T0 P2(2) P2(3) T1 -- so the in-order PE queue never head-of-line
    blocks a ready mm1 behind an xn-gated mm2.
  - P1(b): per n-half, its own [64,512] s_ps bank + eT tile at
    partitions 0:64 (no cross-half WAR -> the Tile scheduler cannot
    hoist a DMA-gated later batch ahead of the ready second half, which
    re-throttled HAM). Exp (bias=b/64-scale); 4 e-transposes against
    [I64|1] emit e-natural + row-sums Z in column 64; invz + broadcast
    mul -> a (bf16).
  - P2(b): 8 v-MULTs back-to-back (weight double-buffer stays hot),
    then 8 one-col asum MULTs (~32ns pitch); interleaving them was 2x
    slower. Batch pairs pack v/asum PSUM rows.
  - For the last batch the asum group runs BEFORE the v-MULTs so the
    pair tail is gated only by the final v-MULT, not the asum drain.
  - Tails (DVE-only + one ACT Sqrt): vv = asum*C^T + v_raw fused stt
    (bf16 out); S = sum vv^2 via stt accum_out; q = 8*sqrt(S+eps) on
    ACT -- the kernel's single Exp->Sqrt table switch,
    tile_wait_until-pinned after the model's last Exp (and pair1's
    chain pinned later still, so pair0's post-sqrt ops place first).
    vfb = vv/q in bf16.
  - Output: direct k-major stores (1-2KB runs); host does the tiny
    (64,512) per-batch transpose. THREE separate DRAM tensors (pair0,
    pair1-half0, pair1-half1) on three DMA queues (scalar / gpsimd /
    sync): Tile WAW-chains writes to a shared DRAM tensor, which had
    serialized the stores; split tensors drain in parallel, and pair1's
    first half's store overlaps its second half's scale. No PE output
    transposes, no ACT copies.
"""

import sys

sys.path.insert(0, "/opt/trn_rl_repo")

from contextlib import ExitStack

import numpy as np

import concourse.bacc as bacc
import concourse.tile as tile
from concourse import mybir
from concourse.bass_utils import run_bass_kernel_spmd

F32 = mybir.dt.float32
BF16 = mybir.dt.bfloat16
FP8 = mybir.dt.float8e3
FP8E4 = mybir.dt.float8e4
AX = mybir.AxisListType
ACTF = mybir.ActivationFunctionType

B_PER_CORE = 4  # 32 batches / 8 cores
N = 1024  # H*W pixels per batch
D = 512
K = 64
EPS = 1e-12
N_CORES = 8
N_WARM = 8


def build_kernel():
    nc = bacc.Bacc()
    xt8_d = nc.declare_dram_parameter("xt8", [128, 4, 2, 2, 2, 512], FP8E4, isOutput=False)
    xn8_d = nc.declare_dram_parameter("xn8", [128, 4, 2, 4, 512], FP8, isOutput=False)
    wkb_d = nc.declare_dram_parameter("wkb", [128, 2, 2, K], FP8E4, isOutput=False)
    b2_d = nc.declare_dram_parameter("b2", [128, 1], F32, isOutput=False)
    ct2_d = nc.declare_dram_parameter("ct2", [128, D], F32, isOutput=False)
    # three separate output tensors: Tile tracks DRAM writes per-tensor,
    # so stores to one shared tensor get WAW-chained and drain serially.
    out0_d = nc.declare_dram_parameter("out0", [128, D], BF16, isOutput=True)
    out1a_d = nc.declare_dram_parameter("out1a", [128, 256], BF16, isOutput=True)
    out1b_d = nc.declare_dram_parameter("out1b", [128, 256], BF16, isOutput=True)

    with tile.TileContext(nc) as tc, ExitStack() as ctx:
        const = ctx.enter_context(tc.tile_pool(name="const", bufs=1))
        xin = ctx.enter_context(tc.tile_pool(name="xin", bufs=1))
        sb = ctx.enter_context(tc.tile_pool(name="sb", bufs=3))
        nrm = ctx.enter_context(tc.tile_pool(name="nrm", bufs=2))
        ps_s = ctx.enter_context(tc.tile_pool(name="ps_s", bufs=3, space="PSUM"))
        ps_e = ctx.enter_context(tc.tile_pool(name="ps_e", bufs=2, space="PSUM"))
        ps_v = ctx.enter_context(tc.tile_pool(name="ps_v", bufs=2, space="PSUM"))
        ps_as = ctx.enter_context(tc.tile_pool(name="ps_as", bufs=1, space="PSUM"))

        # ---- x loads first on the sync queue: xt stays ahead of xn ----
        xt_all = xin.tile([128, 4, 2, 2, 2, 512], FP8E4)
        xn_all = xin.tile([128, 4, 2, 4, 512], FP8)
        # every x tile ships as half-batch (256KB) descriptors: each
        # mm1/mm2 phase pends on a quarter-size transfer, which cuts the
        # per-phase stall when the HBM wire ramps slowly (shared-device
        # contention); xt[b+1] halves stay ordered ahead of xn[b].
        nc.sync.dma_start(out=xt_all[:, 0, 0], in_=xt8_d[:, 0, 0])
        nc.sync.dma_start(out=xt_all[:, 0, 1], in_=xt8_d[:, 0, 1])
        nc.sync.dma_start(out=xt_all[:, 1, 0], in_=xt8_d[:, 1, 0])
        nc.sync.dma_start(out=xt_all[:, 1, 1], in_=xt8_d[:, 1, 1])
        nc.sync.dma_start(out=xn_all[:, 0, 0], in_=xn8_d[:, 0, 0])
        nc.sync.dma_start(out=xn_all[:, 0, 1], in_=xn8_d[:, 0, 1])
        nc.sync.dma_start(out=xt_all[:, 2, 0], in_=xt8_d[:, 2, 0])
        nc.sync.dma_start(out=xt_all[:, 2, 1], in_=xt8_d[:, 2, 1])
        nc.sync.dma_start(out=xn_all[:, 1, 0], in_=xn8_d[:, 1, 0])
        nc.sync.dma_start(out=xn_all[:, 1, 1], in_=xn8_d[:, 1, 1])
        nc.sync.dma_start(out=xt_all[:, 3, 0], in_=xt8_d[:, 3, 0])
        nc.sync.dma_start(out=xt_all[:, 3, 1], in_=xt8_d[:, 3, 1])
        nc.sync.dma_start(out=xn_all[:, 2, 0], in_=xn8_d[:, 2, 0])
        nc.sync.dma_start(out=xn_all[:, 2, 1], in_=xn8_d[:, 2, 1])
        # xn3 ships as quarter-batch (128KB) descriptors: the tail waits
        # on the LAST arrival, and halving the final transfer halves the
        # v-MULT work still pending when the wire finishes
        nc.sync.dma_start(out=xn_all[:, 3, 0, 0:2], in_=xn8_d[:, 3, 0, 0:2])
        nc.sync.dma_start(out=xn_all[:, 3, 0, 2:4], in_=xn8_d[:, 3, 0, 2:4])
        nc.sync.dma_start(out=xn_all[:, 3, 1, 0:2], in_=xn8_d[:, 3, 1, 0:2])
        nc.sync.dma_start(out=xn_all[:, 3, 1, 2:4], in_=xn8_d[:, 3, 1, 2:4])

        # ---- scalar queue: tiny consts, then b0's xt halves (the
        # scalar preamble retires ~1us before sync's, so these bytes are
        # the first to land and drain in parallel with the sync queue);
        # ct2 (256KB, needed only by the pair tails) goes last ----
        wkb = const.tile([128, 2, 2, K], FP8E4)
        nc.scalar.dma_start(out=wkb[:], in_=wkb_d[:])
        b2 = const.tile([128, 1], F32)
        nc.scalar.dma_start(out=b2[:], in_=b2_d[:])
        ct2 = const.tile([128, D], F32)
        nc.scalar.dma_start(out=ct2[:], in_=ct2_d[:])

        # ---- PE warmup: ~3.4us of cold-clock matmuls releases the HAM
        # clock gate right as xt[0] lands. garb is memset first on the
        # DVE so the LDW has a ready operand without waiting on DMAs.
        garb = const.tile([128, 512], BF16)
        nc.vector.memset(garb[:], 0.25)
        # warm shares ps_v's "v" tag rotation (bank reused by pair1's v_ps)
        warm = ps_v.tile([128, 512], F32, tag="v", name="warm")
        for _ in range(N_WARM):
            nc.tensor.matmul(
                warm[:], garb[:, 0:128], garb[:], start=True, stop=True,
                skip_group_check=True,
            )

        # ---- small constants; identity built on-chip ----
        ones = const.tile([128, 1], BF16)
        nc.vector.memset(ones[:], 1.0)
        eps64 = const.tile([128, 1], F32)
        nc.vector.memset(eps64[:], float(64 * EPS))
        S_all = const.tile([128, 2], F32)
        it = const.tile([128, 65], F32)
        nc.gpsimd.iota(
            it[:], pattern=[[1, 65]], base=0, channel_multiplier=0,
            allow_small_or_imprecise_dtypes=True,
        )
        pidx = const.tile([128, 1], F32)
        nc.gpsimd.iota(
            pidx[:], pattern=[[0, 1]], base=0, channel_multiplier=1,
            allow_small_or_imprecise_dtypes=True,
        )
        pidx2 = const.tile([128, 1], F32)
        nc.gpsimd.iota(
            pidx2[:], pattern=[[0, 1]], base=-64, channel_multiplier=1,
            allow_small_or_imprecise_dtypes=True,
        )
        id_aug = const.tile([128, 65], BF16)
        nc.vector.tensor_scalar(
            id_aug[0:64, :], it[0:64, :], pidx[0:64, :], None, mybir.AluOpType.is_equal
        )
        nc.vector.tensor_scalar(
            id_aug[64:128, :], it[64:128, :], pidx2[64:128, :], None,
            mybir.AluOpType.is_equal,
        )
        nc.vector.memset(id_aug[:, 64:65], 1.0)

        v2 = {}
        as2 = {}
        vcs = {}

        def phase1(b):
            """mm1 + softmax -> a_sb[b]  (PE: 8x512 MULTs + 8 e-transposes).
            Each n-half gets its own [64,512] s_ps bank and eT tile at
            partitions 0:64 -- no cross-half WAR on a shared tile, so the
            scheduler never hoists a DMA-gated later batch ahead of the
            ready second half (which stalled the PE into a HAM re-throttle)."""
            invz = sb.tile([128, 8], F32, tag="invz")
            a_sb = sb.tile([128, 8, K], BF16, tag=f"a{b}")
            for h in range(2):
                s_ps = ps_s.tile([64, 512], F32, tag="s", name=f"s{b}h{h}", bufs=3)
                eT = sb.tile([64, 512], BF16, tag="eT", name=f"eT{b}h{h}", bufs=3)
                e_ps = ps_e.tile([128, 4, 65], F32, tag="e", bufs=2)
                # DoubleRow fp8: 2 packed weights/cell, 256-deep
                # contraction per MULT -- 2 MULTs per half instead of 4.
                # Wk ships pre-scaled by 64 (e4m3 subnormal floor); the
                # 1/64 folds into the Exp scale.
                for j in range(2):
                    nc.tensor.matmul(
                        s_ps[:],
                        wkb[:, j],
                        xt_all[:, b, h, j],
                        start=(j == 0),
                        stop=(j == 1),
                        perf_mode=mybir.MatmulPerfMode.DoubleRow,
                        skip_group_check=True,
                    )
                nc.scalar.activation(
                    eT[:], s_ps[:], ACTF.Exp, bias=b2[0:64, :], scale=0.015625
                )
                # out = eT_chunk^T @ [I64 | 1]: e natural + row-sums Z in col 64
                for c in range(4):
                    nc.tensor.matmul(
                        e_ps[:, c, :],
                        eT[:, c * 128 : (c + 1) * 128],
                        id_aug[0:64, :],
                        start=True,
                        stop=True,
                        skip_group_check=True,
                    )
                hs = slice(4 * h, 4 * (h + 1))
                nc.vector.reciprocal(invz[:, hs], e_ps[:, :, 64])
                nc.vector.tensor_mul(
                    a_sb[:, hs, :],
                    e_ps[:, :, 0:K],
                    invz[:, hs].broadcast_to([128, 4, K]),
                )
            return a_sb

        def phase2(b, a_sb):
            """mm2 + asum into pair-packed PSUM rows. For the last batch the
            asum group runs FIRST so the pair tail's vv (which needs both
            v_ps and as_ps) is gated only by the final v-MULT."""
            p2, h2 = b // 2, b % 2
            if h2 == 0:
                v2[p2] = ps_v.tile([128, 512], F32, tag="v", name=f"vps{p2}")
                as2[p2] = ps_as.tile([128, 1], F32, tag="as", name=f"as{p2}")
            v_ps = v2[p2]
            as_ps = as2[p2]
            rows = slice(64 * h2, 64 * (h2 + 1))

            def v_group():
                # all 8 v-MULTs back-to-back: weight double-buffer stays hot
                for c8 in range(8):
                    nc.tensor.matmul(
                        v_ps[rows, :],
                        a_sb[:, c8, :],
                        xn_all[:, b, c8 // 4, c8 % 4, :],
                        start=(c8 == 0),
                        stop=(c8 == 7),
                        skip_group_check=True,
                    )

            def as_group():
                # grouped 1-col asum MULTs (re-LDW is cheap; interleaving isn't)
                for c8 in range(8):
                    nc.tensor.matmul(
                        as_ps[rows, :],
                        a_sb[:, c8, :],
                        ones[:],
                        start=(c8 == 0),
                        stop=(c8 == 7),
                        skip_group_check=True,
                    )

            if b == 3:
                as_group()
                # pair1's ct2*asum product precomputes on the DVE while
                # the v-MULTs still run: only the cheap add remains on
                # the post-PE critical path
                vc1 = nrm.tile([128, D], F32, tag="vc1", name="vc1")
                nc.vector.tensor_scalar_mul(vc1[:], ct2[:], as_ps[:, 0:1])
                vcs[1] = vc1
                v_group()
            else:
                v_group()
                as_group()

        def tail_norm(p2):
            """v = asum*C^T + v_raw (fused stt); S = sum_d v^2 via accum_out;
            q = 8*sqrt(S+eps) on ACT (single Exp->Sqrt table switch);
            vfb = v/q. Pins: only SQRT(p0) (just after the model's last
            Exp, so the Sqrt table loads once) and pair1's whole chain
            (later still, so pair0's post-sqrt ops place first on the
            DVE and its store issues early)."""
            v_ps = v2[p2]
            A = mybir.AluOpType
            # bf16 intermediates: 2x DVE rate on the serial pair1 chain;
            # S still accumulates in f32 via accum_out
            vv = nrm.tile([128, D], BF16, tag=f"vv{p2}", name=f"vv{p2}")
            vsq = nrm.tile([128, D], BF16, tag=f"vsq{p2}", name=f"vsq{p2}")
            use_vc = p2 == 1
            q = nrm.tile([128, 1], F32, tag=f"q{p2}", name=f"q{p2}")
            sc = nrm.tile([128, 1], F32, tag=f"sc{p2}", name=f"sc{p2}")
            vfb = nrm.tile([128, D], BF16, tag=f"vfb{p2}", name=f"vfb{p2}")
            with tc.tile_wait_until(0.028, enable=(p2 == 1)):
                if use_vc:
                    nc.vector.tensor_add(vv[:], vcs[1][:], v_ps[:])
                else:
                    nc.vector.scalar_tensor_tensor(
                        vv[:], ct2[:], as2[p2][:, 0:1], v_ps[:], A.mult, A.add
                    )
                nc.vector.scalar_tensor_tensor(
                    vsq[:], vv[:], 1.0, vv[:], A.bypass, A.mult,
                    accum_out=S_all[:, p2 : p2 + 1],
                )
                with tc.tile_wait_until(0.024, enable=(p2 == 0)):
                    nc.scalar.activation(
                        q[:], S_all[:, p2 : p2 + 1], ACTF.Sqrt,
                        bias=eps64[:], scale=64.0,
                    )
                nc.vector.reciprocal(sc[:], q[:])
                if p2 == 1:
                    # split the final scale+store into column halves; each
                    # half has its own DRAM tensor and DMA queue (gpsimd /
                    # sync -- both idle) so the two 64KB stores drain in
                    # parallel and overlap the second half's scale
                    nc.vector.tensor_scalar_mul(vfb[:, 0:256], vv[:, 0:256], sc[:, 0:1])
                    nc.gpsimd.dma_start(out=out1a_d[:], in_=vfb[:, 0:256])
                    nc.vector.tensor_scalar_mul(vfb[:, 256:512], vv[:, 256:512], sc[:, 0:1])
                    nc.sync.dma_start(out=out1b_d[:], in_=vfb[:, 256:512])
                else:
                    nc.vector.tensor_scalar_mul(vfb[:], vv[:], sc[:, 0:1])
            return vfb

        def tail_store(p2, vfb):
            """Direct k-major store (1KB runs); host does the tiny (64,512)
            per-batch transpose. Pair0 goes out on the scalar engine's DMA
            queue so it never waits behind the x input stream; pair1
            stores inline in tail_norm (split halves, two queues)."""
            nc.scalar.dma_start(out=out0_d[:], in_=vfb[:])

        # ---- interleaved schedule: no PE head-of-line blocking ----
        a0 = phase1(0)
        a1 = phase1(1)
        phase2(0, a0)
        a2 = phase1(2)
        phase2(1, a1)
        a3 = phase1(3)
        vfb0 = tail_norm(0)  # DVE-only; fills DVE idle gaps
        tail_store(0, vfb0)
        phase2(2, a2)
        phase2(3, a3)
        tail_norm(1)  # pair1 stores inside (split halves)

    nc.compile()
    return nc


_CACHED_NC = None


def _get_nc():
    global _CACHED_NC
    if _CACHED_NC is None:
        _CACHED_NC = build_kernel()
    return _CACHED_NC


def build_in_maps(x, Wk, b, C):
    import ml_dtypes

    B = x.shape[0]
    x2 = np.ascontiguousarray(x, dtype=np.float32).reshape(B, N, D)
    bpc = B // N_CORES
    Wkf = np.asarray(Wk, dtype=np.float32)
    Cf = np.asarray(C, dtype=np.float32)
    bf = np.asarray(b, dtype=np.float32).reshape(K)
    consts = {
        # d = j*256 + o*128 + p; DoubleRow packs (o) pairs per PE cell.
        # x64 lifts the 0.02-scale weights off the e4m3 subnormal floor.
        "wkb": np.ascontiguousarray(
            (Wkf * 64.0).reshape(2, 2, 128, K).transpose(2, 0, 1, 3)
        ).astype(ml_dtypes.float8_e4m3),
        "ct2": np.ascontiguousarray(np.concatenate([Cf.T, Cf.T], axis=0)),
        "b2": np.concatenate([bf, bf]).reshape(128, 1),
    }
    in_maps = []
    for c in range(N_CORES):
        A = x2[c * bpc : (c + 1) * bpc]  # (4, 1024, 512)
        # xn8[p, b, h, c, d]: pixel n = (4h+c)*128 + p
        xn8 = np.ascontiguousarray(
            A.reshape(bpc, 2, 4, 128, D).transpose(3, 0, 1, 2, 4)
        ).astype(ml_dtypes.float8_e3m4)
        # xt8[p, b, h, j, o, nn]: d = j*256 + o*128 + p, n = h*512 + nn
        xt8 = np.ascontiguousarray(
            A.transpose(0, 2, 1)
            .reshape(bpc, 2, 2, 128, 2, 512)
            .transpose(3, 0, 4, 1, 2, 5)
        ).astype(ml_dtypes.float8_e4m3)
        in_maps.append({"xn8": xn8, "xt8": xt8, **consts})
    return in_maps


def kernel(x, Wk, b, C):
    """Full-input NetVLAD forward. x (32,32,32,512) f32 -> out (32, 32768) f32."""
    in_maps = build_in_maps(x, Wk, b, C)
    nc = _get_nc()
    res = run_bass_kernel_spmd(nc, in_maps, list(range(N_CORES)))
    outs = []
    for c in range(N_CORES):
        o = np.stack(
            [
                np.asarray(res.results[c]["out0"]),
                np.concatenate(
                    [
                        np.asarray(res.results[c]["out1a"]),
                        np.asarray(res.results[c]["out1b"]),
                    ],
                    axis=1,
                ),
            ],
            axis=1,
        )  # (128, 2, 512) bf16: p=64*hh+k
        outs.append(
            o.reshape(2, K, 2, D)
            .transpose(2, 0, 3, 1)
            .reshape(B_PER_CORE, D * K)
            .astype(np.float32)
        )
    return np.concatenate(outs, axis=0)


# revision 48
# speedup vs baseline: 1.2464x; 1.0070x over previous
"""NetVLAD forward kernel for 8 TRN2 NeuronCores (Bass/Tile).

Reference (per batch b of 32):
  s = x @ Wk + b         (1024, 64) logits;  softmax over k -> a
  v[d,k] = sum_n a[n,k] x[n,d] + (sum_n a[n,k]) * C[d,k]
  v /= ||v||_2 over d (per k);  out = flatten(v) / ||flatten(v)||_2

Sharding: data-parallel over batch B=32 across 8 cores (4 batches/core).
Wk, b, C replicated; no collectives; host concatenates outputs.

Design (v8) -- measured 34.9us best, 37-41us on slow-HBM-contention
runs, vs 44.8us baseline:
  - x ships twice in fp8 (4MB/core): xt e4m3 (d on partitions, mm1) +
    xn e3m4 (pixels on partitions, mm2). Half-batch (256KB) descriptors
    on the sync queue, xt[b+1] ordered ahead of xn[b], so every phase
    pends on a small transfer -- cushions slow HBM-contention runs; the
    last tile (xn3) ships as 128KB quarters so the tail's final v-MULTs
    pend on the smallest possible arrival. Total HW rel-err 1.39e-2 vs
    the 2e-2 gate (deterministic inputs).
  - mm1 runs fp8e4m3 DoubleRow (2 packed weights/cell, 256-deep
    contraction): 2 MULTs per n-half instead of 4, ~1.44x PE win at
    FD=512. Wk ships pre-scaled x64 (off the e4m3 subnormal floor);
    the 1/64 folds into the Exp activation scale.
  - Warmup: 8 garbage 512-col matmuls (~3.4us at the cold 1.2GHz clock,
    the HAM activity budget) so the clock gate releases ~when xt0 lands.
  - Emission interleaves phases -- P1(0) P1(1) P2(0) P1(2) P2(1) P1(3)
    T0 P2(2) P2(3) T1 -- so the in-order PE queue never head-of-line
    blocks a ready mm1 behind an xn-gated mm2.
  - P1(b): per n-half, its own [64,512] s_ps bank + eT tile at
    partitions 0:64 (no cross-half WAR -> the Tile scheduler cannot
    hoist a DMA-gated later batch ahead of the ready second half, which
    re-throttled HAM). Exp (bias=b/64-scale); 4 e-transposes against
    [I64|1] emit e-natural + row-sums Z in column 64; invz + broadcast
    mul -> a (bf16).
  - P2(b): 8 v-MULTs back-to-back (weight double-buffer stays hot),
    then 8 one-col asum MULTs (~32ns pitch); interleaving them was 2x
    slower. Batch pairs pack v/asum PSUM rows.
  - For the last batch the asum group runs BEFORE the v-MULTs so the
    pair tail is gated only by the final v-MULT, not the asum drain.
  - Tails (DVE-only + one ACT Sqrt): vv = asum*C^T + v_raw (for pair1
    the ct2*asum product precomputes on the DVE during the v-MULTs --
    asum-first makes as_ps available early -- so only an add remains
    post-PE; pair0 uses the fused stt); S = sum vv^2 via stt accum_out; q = 8*sqrt(S+eps) on
    ACT -- the kernel's single Exp->Sqrt table switch,
    tile_wait_until-pinned after the model's last Exp (and pair1's
    chain pinned later still, so pair0's post-sqrt ops place first).
    vfb = vv/q in bf16.
  - Output: direct k-major stores (1-2KB runs); host does the tiny
    (64,512) per-batch transpose. THREE separate DRAM tensors (pair0,
    pair1-half0, pair1-half1) on three DMA queues (scalar / gpsimd /
    sync): Tile WAW-chains writes to a shared DRAM tensor, which had
    serialized the stores; split tensors drain in parallel, and pair1's
    first half's store overlaps its second half's scale. No PE output
    transposes, no ACT copies.
"""

import sys

sys.path.insert(0, "/opt/trn_rl_repo")

from contextlib import ExitStack

import numpy as np

import concourse.bacc as bacc
import concourse.tile as tile
from concourse import mybir
from concourse.bass_utils import run_bass_kernel_spmd

F32 = mybir.dt.float32
BF16 = mybir.dt.bfloat16
FP8 = mybir.dt.float8e3
FP8E4 = mybir.dt.float8e4
AX = mybir.AxisListType
ACTF = mybir.ActivationFunctionType

B_PER_CORE = 4  # 32 batches / 8 cores
N = 1024  # H*W pixels per batch
D = 512
K = 64
EPS = 1e-12
N_CORES = 8
N_WARM = 8


def build_kernel():
    nc = bacc.Bacc()
    xt8_d = nc.declare_dram_parameter("xt8", [128, 4, 2, 2, 2, 512], FP8E4, isOutput=False)
    xn8_d = nc.declare_dram_parameter("xn8", [128, 4, 2, 4, 512], FP8, isOutput=False)
    wkb_d = nc.declare_dram_parameter("wkb", [128, 2, 2, K], FP8E4, isOutput=False)
    b2_d = nc.declare_dram_parameter("b2", [128, 1], F32, isOutput=False)
    ct2_d = nc.declare_dram_parameter("ct2", [128, D], F32, isOutput=False)
    # three separate output tensors: Tile tracks DRAM writes per-tensor,
    # so stores to one shared tensor get WAW-chained and drain serially.
    out0_d = nc.declare_dram_parameter("out0", [128, D], BF16, isOutput=True)
    out1a_d = nc.declare_dram_parameter("out1a", [128, 256], BF16, isOutput=True)
    out1b_d = nc.declare_dram_parameter("out1b", [128, 256], BF16, isOutput=True)

    with tile.TileContext(nc) as tc, ExitStack() as ctx:
        const = ctx.enter_context(tc.tile_pool(name="const", bufs=1))
        xin = ctx.enter_context(tc.tile_pool(name="xin", bufs=1))
        sb = ctx.enter_context(tc.tile_pool(name="sb", bufs=3))
        nrm = ctx.enter_context(tc.tile_pool(name="nrm", bufs=2))
        ps_s = ctx.enter_context(tc.tile_pool(name="ps_s", bufs=3, space="PSUM"))
        ps_e = ctx.enter_context(tc.tile_pool(name="ps_e", bufs=2, space="PSUM"))
        ps_v = ctx.enter_context(tc.tile_pool(name="ps_v", bufs=2, space="PSUM"))
        ps_as = ctx.enter_context(tc.tile_pool(name="ps_as", bufs=1, space="PSUM"))

        # ---- x loads first on the sync queue: xt stays ahead of xn ----
        xt_all = xin.tile([128, 4, 2, 2, 2, 512], FP8E4)
        xn_all = xin.tile([128, 4, 2, 4, 512], FP8)
        # every x tile ships as half-batch (256KB) descriptors: each
        # mm1/mm2 phase pends on a quarter-size transfer, which cuts the
        # per-phase stall when the HBM wire ramps slowly (shared-device
        # contention); xt[b+1] halves stay ordered ahead of xn[b].
        nc.sync.dma_start(out=xt_all[:, 0, 0], in_=xt8_d[:, 0, 0])
        nc.sync.dma_start(out=xt_all[:, 0, 1], in_=xt8_d[:, 0, 1])
        nc.sync.dma_start(out=xt_all[:, 1, 0], in_=xt8_d[:, 1, 0])
        nc.sync.dma_start(out=xt_all[:, 1, 1], in_=xt8_d[:, 1, 1])
        nc.sync.dma_start(out=xn_all[:, 0, 0], in_=xn8_d[:, 0, 0])
        nc.sync.dma_start(out=xn_all[:, 0, 1], in_=xn8_d[:, 0, 1])
        nc.sync.dma_start(out=xt_all[:, 2, 0], in_=xt8_d[:, 2, 0])
        nc.sync.dma_start(out=xt_all[:, 2, 1], in_=xt8_d[:, 2, 1])
        nc.sync.dma_start(out=xn_all[:, 1, 0], in_=xn8_d[:, 1, 0])
        nc.sync.dma_start(out=xn_all[:, 1, 1], in_=xn8_d[:, 1, 1])
        nc.sync.dma_start(out=xt_all[:, 3, 0], in_=xt8_d[:, 3, 0])
        nc.sync.dma_start(out=xt_all[:, 3, 1], in_=xt8_d[:, 3, 1])
        nc.sync.dma_start(out=xn_all[:, 2, 0], in_=xn8_d[:, 2, 0])
        nc.sync.dma_start(out=xn_all[:, 2, 1], in_=xn8_d[:, 2, 1])
        # xn3 ships as quarter-batch (128KB) descriptors: the tail waits
        # on the LAST arrival, and halving the final transfer halves the
        # v-MULT work still pending when the wire finishes
        nc.sync.dma_start(out=xn_all[:, 3, 0, 0:2], in_=xn8_d[:, 3, 0, 0:2])
        nc.sync.dma_start(out=xn_all[:, 3, 0, 2:4], in_=xn8_d[:, 3, 0, 2:4])
        nc.sync.dma_start(out=xn_all[:, 3, 1, 0:2], in_=xn8_d[:, 3, 1, 0:2])
        nc.sync.dma_start(out=xn_all[:, 3, 1, 2:4], in_=xn8_d[:, 3, 1, 2:4])

        # ---- scalar queue: tiny consts, then b0's xt halves (the
        # scalar preamble retires ~1us before sync's, so these bytes are
        # the first to land and drain in parallel with the sync queue);
        # ct2 (256KB, needed only by the pair tails) goes last ----
        wkb = const.tile([128, 2, 2, K], FP8E4)
        nc.scalar.dma_start(out=wkb[:], in_=wkb_d[:])
        b2 = const.tile([128, 1], F32)
        nc.scalar.dma_start(out=b2[:], in_=b2_d[:])
        ct2 = const.tile([128, D], F32)
        nc.scalar.dma_start(out=ct2[:], in_=ct2_d[:])

        # ---- PE warmup: ~3.4us of cold-clock matmuls releases the HAM
        # clock gate right as xt[0] lands. garb is memset first on the
        # DVE so the LDW has a ready operand without waiting on DMAs.
        garb = const.tile([128, 512], BF16)
        nc.vector.memset(garb[:], 0.25)
        # warm shares ps_v's "v" tag rotation (bank reused by pair1's v_ps)
        warm = ps_v.tile([128, 512], F32, tag="v", name="warm")
        for _ in range(N_WARM):
            nc.tensor.matmul(
                warm[:], garb[:, 0:128], garb[:], start=True, stop=True,
                skip_group_check=True,
            )

        # ---- small constants; identity built on-chip ----
        ones = const.tile([128, 1], BF16)
        nc.vector.memset(ones[:], 1.0)
        eps64 = const.tile([128, 1], F32)
        nc.vector.memset(eps64[:], float(64 * EPS))
        S_all = const.tile([128, 2], F32)
        it = const.tile([128, 65], F32)
        nc.gpsimd.iota(
            it[:], pattern=[[1, 65]], base=0, channel_multiplier=0,
            allow_small_or_imprecise_dtypes=True,
        )
        pidx = const.tile([128, 1], F32)
        nc.gpsimd.iota(
            pidx[:], pattern=[[0, 1]], base=0, channel_multiplier=1,
            allow_small_or_imprecise_dtypes=True,
        )
        pidx2 = const.tile([128, 1], F32)
        nc.gpsimd.iota(
            pidx2[:], pattern=[[0, 1]], base=-64, channel_multiplier=1,
            allow_small_or_imprecise_dtypes=True,
        )
        id_aug = const.tile([128, 65], BF16)
        nc.vector.tensor_scalar(
            id_aug[0:64, :], it[0:64, :], pidx[0:64, :], None, mybir.AluOpType.is_equal
        )
        nc.vector.tensor_scalar(
            id_aug[64:128, :], it[64:128, :], pidx2[64:128, :], None,
            mybir.AluOpType.is_equal,
        )
        nc.vector.memset(id_aug[:, 64:65], 1.0)

        v2 = {}
        as2 = {}
        vcs = {}

        def phase1(b):
            """mm1 + softmax -> a_sb[b]  (PE: 8x512 MULTs + 8 e-transposes).
            Each n-half gets its own [64,512] s_ps bank and eT tile at
            partitions 0:64 -- no cross-half WAR on a shared tile, so the
            scheduler never hoists a DMA-gated later batch ahead of the
            ready second half (which stalled the PE into a HAM re-throttle)."""
            invz = sb.tile([128, 8], F32, tag="invz")
            a_sb = sb.tile([128, 8, K], BF16, tag=f"a{b}")
            for h in range(2):
                s_ps = ps_s.tile([64, 512], F32, tag="s", name=f"s{b}h{h}", bufs=3)
                eT = sb.tile([64, 512], BF16, tag="eT", name=f"eT{b}h{h}", bufs=3)
                e_ps = ps_e.tile([128, 4, 65], F32, tag="e", bufs=2)
                # DoubleRow fp8: 2 packed weights/cell, 256-deep
                # contraction per MULT -- 2 MULTs per half instead of 4.
                # Wk ships pre-scaled by 64 (e4m3 subnormal floor); the
                # 1/64 folds into the Exp scale.
                for j in range(2):
                    nc.tensor.matmul(
                        s_ps[:],
                        wkb[:, j],
                        xt_all[:, b, h, j],
                        start=(j == 0),
                        stop=(j == 1),
                        perf_mode=mybir.MatmulPerfMode.DoubleRow,
                        skip_group_check=True,
                    )
                nc.scalar.activation(
                    eT[:], s_ps[:], ACTF.Exp, bias=b2[0:64, :], scale=0.015625
                )
                # out = eT_chunk^T @ [I64 | 1]: e natural + row-sums Z in col 64
                for c in range(4):
                    nc.tensor.matmul(
                        e_ps[:, c, :],
                        eT[:, c * 128 : (c + 1) * 128],
                        id_aug[0:64, :],
                        start=True,
                        stop=True,
                        skip_group_check=True,
                    )
                hs = slice(4 * h, 4 * (h + 1))
                nc.vector.reciprocal(invz[:, hs], e_ps[:, :, 64])
                nc.vector.tensor_mul(
                    a_sb[:, hs, :],
                    e_ps[:, :, 0:K],
                    invz[:, hs].broadcast_to([128, 4, K]),
                )
            return a_sb

        def phase2(b, a_sb):
            """mm2 + asum into pair-packed PSUM rows. For the last batch the
            asum group runs FIRST so the pair tail's vv (which needs both
            v_ps and as_ps) is gated only by the final v-MULT."""
            p2, h2 = b // 2, b % 2
            if h2 == 0:
                v2[p2] = ps_v.tile([128, 512], F32, tag="v", name=f"vps{p2}")
                as2[p2] = ps_as.tile([128, 1], F32, tag="as", name=f"as{p2}")
            v_ps = v2[p2]
            as_ps = as2[p2]
            rows = slice(64 * h2, 64 * (h2 + 1))

            def v_group():
                # all 8 v-MULTs back-to-back: weight double-buffer stays hot
                for c8 in range(8):
                    nc.tensor.matmul(
                        v_ps[rows, :],
                        a_sb[:, c8, :],
                        xn_all[:, b, c8 // 4, c8 % 4, :],
                        start=(c8 == 0),
                        stop=(c8 == 7),
                        skip_group_check=True,
                    )

            def as_group():
                # grouped 1-col asum MULTs (re-LDW is cheap; interleaving isn't)
                for c8 in range(8):
                    nc.tensor.matmul(
                        as_ps[rows, :],
                        a_sb[:, c8, :],
                        ones[:],
                        start=(c8 == 0),
                        stop=(c8 == 7),
                        skip_group_check=True,
                    )

            if b == 3:
                as_group()
                # pair1's ct2*asum product precomputes on the DVE while
                # the v-MULTs still run: only the cheap add remains on
                # the post-PE critical path
                vc1 = nrm.tile([128, D], F32, tag="vc1", name="vc1")
                nc.vector.tensor_scalar_mul(vc1[:], ct2[:], as_ps[:, 0:1])
                vcs[1] = vc1
                v_group()
            else:
                v_group()
                as_group()

        def tail_norm(p2):
            """v = asum*C^T + v_raw (fused stt); S = sum_d v^2 via accum_out;
            q = 8*sqrt(S+eps) on ACT (single Exp->Sqrt table switch);
            vfb = v/q. Pins: only SQRT(p0) (just after the model's last
            Exp, so the Sqrt table loads once) and pair1's whole chain
            (later still, so pair0's post-sqrt ops place first on the
            DVE and its store issues early)."""
            v_ps = v2[p2]
            A = mybir.AluOpType
            # bf16 intermediates: 2x DVE rate on the serial pair1 chain;
            # S still accumulates in f32 via accum_out
            vv = nrm.tile([128, D], BF16, tag=f"vv{p2}", name=f"vv{p2}")
            vsq = nrm.tile([128, D], BF16, tag=f"vsq{p2}", name=f"vsq{p2}")
            use_vc = p2 == 1
            q = nrm.tile([128, 1], F32, tag=f"q{p2}", name=f"q{p2}")
            sc = nrm.tile([128, 1], F32, tag=f"sc{p2}", name=f"sc{p2}")
            vfb = nrm.tile([128, D], BF16, tag=f"vfb{p2}", name=f"vfb{p2}")
            with tc.tile_wait_until(0.028, enable=(p2 == 1)):
                if use_vc:
                    nc.vector.tensor_add(vv[:], vcs[1][:], v_ps[:])
                else:
                    nc.vector.scalar_tensor_tensor(
                        vv[:], ct2[:], as2[p2][:, 0:1], v_ps[:], A.mult, A.add
                    )
                nc.vector.scalar_tensor_tensor(
                    vsq[:], vv[:], 1.0, vv[:], A.bypass, A.mult,
                    accum_out=S_all[:, p2 : p2 + 1],
                )
                with tc.tile_wait_until(0.024, enable=(p2 == 0)):
                    nc.scalar.activation(
                        q[:], S_all[:, p2 : p2 + 1], ACTF.Sqrt,
                        bias=eps64[:], scale=64.0,
                    )
                nc.vector.reciprocal(sc[:], q[:])
                if p2 == 1:
                    # split the final scale+store into column halves; each
                    # half has its own DRAM tensor and DMA queue (gpsimd /
                    # sync -- both idle) so the two 64KB stores drain in
                    # parallel and overlap the second half's scale
                    nc.vector.tensor_scalar_mul(vfb[:, 0:256], vv[:, 0:256], sc[:, 0:1])
                    nc.gpsimd.dma_start(out=out1a_d[:], in_=vfb[:, 0:256])
                    nc.vector.tensor_scalar_mul(vfb[:, 256:512], vv[:, 256:512], sc[:, 0:1])
                    nc.sync.dma_start(out=out1b_d[:], in_=vfb[:, 256:512])
                else:
                    nc.vector.tensor_scalar_mul(vfb[:], vv[:], sc[:, 0:1])
            return vfb

        def tail_store(p2, vfb):
            """Direct k-major store (1KB runs); host does the tiny (64,512)
            per-batch transpose. Pair0 goes out on the scalar engine's DMA
            queue so it never waits behind the x input stream; pair1
            stores inline in tail_norm (split halves, two queues)."""
            nc.scalar.dma_start(out=out0_d[:], in_=vfb[:])

        # ---- interleaved schedule: no PE head-of-line blocking ----
        a0 = phase1(0)
        a1 = phase1(1)
        phase2(0, a0)
        a2 = phase1(2)
        phase2(1, a1)
        a3 = phase1(3)
        vfb0 = tail_norm(0)  # DVE-only; fills DVE idle gaps
        tail_store(0, vfb0)
        phase2(2, a2)
        phase2(3, a3)
        tail_norm(1)  # pair1 stores inside (split halves)

    nc.compile()
    return nc


_CACHED_NC = None


def _get_nc():
    global _CACHED_NC
    if _CACHED_NC is None:
        _CACHED_NC = build_kernel()
    return _CACHED_NC


def build_in_maps(x, Wk, b, C):
    import ml_dtypes

    B = x.shape[0]
    x2 = np.ascontiguousarray(x, dtype=np.float32).reshape(B, N, D)
    bpc = B // N_CORES
    Wkf = np.asarray(Wk, dtype=np.float32)
    Cf = np.asarray(C, dtype=np.float32)
    bf = np.asarray(b, dtype=np.float32).reshape(K)
    consts = {
        # d = j*256 + o*128 + p; DoubleRow packs (o) pairs per PE cell.
        # x64 lifts the 0.02-scale weights off the e4m3 subnormal floor.
        "wkb": np.ascontiguousarray(
            (Wkf * 64.0).reshape(2, 2, 128, K).transpose(2, 0, 1, 3)
        ).astype(ml_dtypes.float8_e4m3),
        "ct2": np.ascontiguousarray(np.concatenate([Cf.T, Cf.T], axis=0)),
        "b2": np.concatenate([bf, bf]).reshape(128, 1),
    }
    in_maps = []
    for c in range(N_CORES):
        A = x2[c * bpc : (c + 1) * bpc]  # (4, 1024, 512)
        # xn8[p, b, h, c, d]: pixel n = (4h+c)*128 + p
        xn8 = np.ascontiguousarray(
            A.reshape(bpc, 2, 4, 128, D).transpose(3, 0, 1, 2, 4)
        ).astype(ml_dtypes.float8_e3m4)
        # xt8[p, b, h, j, o, nn]: d = j*256 + o*128 + p, n = h*512 + nn
        xt8 = np.ascontiguousarray(
            A.transpose(0, 2, 1)
            .reshape(bpc, 2, 2, 128, 2, 512)
            .transpose(3, 0, 4, 1, 2, 5)
        ).astype(ml_dtypes.float8_e4m3)
        in_maps.append({"xn8": xn8, "xt8": xt8, **consts})
    return in_maps


def kernel(x, Wk, b, C):
    """Full-input NetVLAD forward. x (32,32,32,512) f32 -> out (32, 32768) f32."""
    in_maps = build_in_maps(x, Wk, b, C)
    nc = _get_nc()
    res = run_bass_kernel_spmd(nc, in_maps, list(range(N_CORES)))
    outs = []
    for c in range(N_CORES):
        o = np.stack(
            [
                np.asarray(res.results[c]["out0"]),
                np.concatenate(
                    [
                        np.asarray(res.results[c]["out1a"]),
                        np.asarray(res.results[c]["out1b"]),
                    ],
                    axis=1,
                ),
            ],
            axis=1,
        )  # (128, 2, 512) bf16: p=64*hh+k
        outs.append(
            o.reshape(2, K, 2, D)
            .transpose(2, 0, 3, 1)
            .reshape(B_PER_CORE, D * K)
            .astype(np.float32)
        )
    return np.concatenate(outs, axis=0)


# revision 49
# speedup vs baseline: 1.2607x; 1.0115x over previous
"""NetVLAD forward kernel for 8 TRN2 NeuronCores (Bass/Tile).

Reference (per batch b of 32):
  s = x @ Wk + b         (1024, 64) logits;  softmax over k -> a
  v[d,k] = sum_n a[n,k] x[n,d] + (sum_n a[n,k]) * C[d,k]
  v /= ||v||_2 over d (per k);  out = flatten(v) / ||flatten(v)||_2

Sharding: data-parallel over batch B=32 across 8 cores (4 batches/core).
Wk, b, C replicated; no collectives; host concatenates outputs.

Design (v8) -- measured 34.9us best, 37-41us on slow-HBM-contention
runs, vs 44.8us baseline:
  - x ships twice in fp8 (4MB/core): xt e4m3 (d on partitions, mm1) +
    xn e3m4 (pixels on partitions, mm2). Half-batch (256KB) descriptors
    on the sync queue, xt[b+1] ordered ahead of xn[b], so every phase
    pends on a small transfer -- cushions slow HBM-contention runs; the
    last tile (xn3) ships as 128KB quarters so the tail's final v-MULTs
    pend on the smallest possible arrival. Total HW rel-err 1.39e-2 vs
    the 2e-2 gate (deterministic inputs).
  - mm1 runs fp8e4m3 DoubleRow (2 packed weights/cell, 256-deep
    contraction): 2 MULTs per n-half instead of 4, ~1.44x PE win at
    FD=512. Wk ships pre-scaled x64 (off the e4m3 subnormal floor);
    the 1/64 folds into the Exp activation scale.
  - Warmup: 8 garbage 512-col matmuls (~3.4us at the cold 1.2GHz clock,
    the HAM activity budget) so the clock gate releases ~when xt0 lands.
  - Emission interleaves phases -- P1(0) P1(1) P2(0) P1(2) P2(1) P1(3)
    T0 P2(2) P2(3) T1 -- so the in-order PE queue never head-of-line
    blocks a ready mm1 behind an xn-gated mm2.
  - P1(b): per n-half, its own [64,512] s_ps bank + eT tile at
    partitions 0:64 (no cross-half WAR -> the Tile scheduler cannot
    hoist a DMA-gated later batch ahead of the ready second half, which
    re-throttled HAM). Exp (bias=b/64-scale); 4 e-transposes against
    [I64|1] emit e-natural + row-sums Z in column 64; invz + broadcast
    mul -> a (bf16).
  - P2(b): 8 v-MULTs back-to-back (weight double-buffer stays hot),
    then 8 one-col asum MULTs (~32ns pitch); interleaving them was 2x
    slower. Batch pairs pack v/asum PSUM rows.
  - For the last batch the asum group runs BEFORE the v-MULTs so the
    pair tail is gated only by the final v-MULT, not the asum drain.
  - Tails (DVE-only + one ACT Sqrt): vv = asum*C^T + v_raw (for pair1
    the ct2*asum product precomputes on the DVE during the v-MULTs --
    asum-first makes as_ps available early -- so only an add remains
    post-PE; pair0 uses the fused stt); S = sum vv^2 via stt accum_out; q = 8*sqrt(S+eps) on
    ACT -- the kernel's single Exp->Sqrt table switch,
    tile_wait_until-pinned after the model's last Exp (and pair1's
    chain pinned later still, so pair0's post-sqrt ops place first).
    vfb = vv/q in bf16.
  - Output: direct k-major stores (1-2KB runs); host does the tiny
    (64,512) per-batch transpose. THREE separate DRAM tensors (pair0,
    pair1-half0, pair1-half1) on three DMA queues (scalar / gpsimd /
    sync): Tile WAW-chains writes to a shared DRAM tensor, which had
    serialized the stores; split tensors drain in parallel, and pair1's
    first half's store overlaps its second half's scale. No PE output
    transposes, no ACT copies.
"""

import sys

sys.path.insert(0, "/opt/trn_rl_repo")

from contextlib import ExitStack

import numpy as np

import concourse.bacc as bacc
import concourse.tile as tile
from concourse import mybir
from concourse.bass_utils import run_bass_kernel_spmd

F32 = mybir.dt.float32
BF16 = mybir.dt.bfloat16
FP8 = mybir.dt.float8e3
FP8E4 = mybir.dt.float8e4
AX = mybir.AxisListType
ACTF = mybir.ActivationFunctionType

B_PER_CORE = 4  # 32 batches / 8 cores
N = 1024  # H*W pixels per batch
D = 512
K = 64
EPS = 1e-12
N_CORES = 8
N_WARM = 8


def build_kernel():
    nc = bacc.Bacc()
    xt8_d = nc.declare_dram_parameter("xt8", [128, 4, 2, 2, 2, 512], FP8E4, isOutput=False)
    xn8_d = nc.declare_dram_parameter("xn8", [128, 4, 2, 4, 512], FP8, isOutput=False)
    wkb_d = nc.declare_dram_parameter("wkb", [128, 2, 2, K], FP8E4, isOutput=False)
    b2_d = nc.declare_dram_parameter("b2", [128, 1], F32, isOutput=False)
    ct2_d = nc.declare_dram_parameter("ct2", [128, D], F32, isOutput=False)
    # three separate output tensors: Tile tracks DRAM writes per-tensor,
    # so stores to one shared tensor get WAW-chained and drain serially.
    out0_d = nc.declare_dram_parameter("out0", [128, D], BF16, isOutput=True)
    out1q = [
        nc.declare_dram_parameter(f"out1{u}", [128, 128], BF16, isOutput=True)
        for u in range(4)
    ]

    with tile.TileContext(nc) as tc, ExitStack() as ctx:
        const = ctx.enter_context(tc.tile_pool(name="const", bufs=1))
        xin = ctx.enter_context(tc.tile_pool(name="xin", bufs=1))
        sb = ctx.enter_context(tc.tile_pool(name="sb", bufs=3))
        nrm = ctx.enter_context(tc.tile_pool(name="nrm", bufs=2))
        ps_s = ctx.enter_context(tc.tile_pool(name="ps_s", bufs=3, space="PSUM"))
        ps_e = ctx.enter_context(tc.tile_pool(name="ps_e", bufs=2, space="PSUM"))
        ps_v = ctx.enter_context(tc.tile_pool(name="ps_v", bufs=2, space="PSUM"))
        ps_as = ctx.enter_context(tc.tile_pool(name="ps_as", bufs=1, space="PSUM"))

        # ---- x loads first on the sync queue: xt stays ahead of xn ----
        xt_all = xin.tile([128, 4, 2, 2, 2, 512], FP8E4)
        xn_all = xin.tile([128, 4, 2, 4, 512], FP8)
        # every x tile ships as half-batch (256KB) descriptors: each
        # mm1/mm2 phase pends on a quarter-size transfer, which cuts the
        # per-phase stall when the HBM wire ramps slowly (shared-device
        # contention); xt[b+1] halves stay ordered ahead of xn[b].
        nc.sync.dma_start(out=xt_all[:, 0, 0], in_=xt8_d[:, 0, 0])
        nc.sync.dma_start(out=xt_all[:, 0, 1], in_=xt8_d[:, 0, 1])
        nc.sync.dma_start(out=xt_all[:, 1, 0], in_=xt8_d[:, 1, 0])
        nc.sync.dma_start(out=xt_all[:, 1, 1], in_=xt8_d[:, 1, 1])
        nc.sync.dma_start(out=xn_all[:, 0, 0], in_=xn8_d[:, 0, 0])
        nc.sync.dma_start(out=xn_all[:, 0, 1], in_=xn8_d[:, 0, 1])
        nc.sync.dma_start(out=xt_all[:, 2, 0], in_=xt8_d[:, 2, 0])
        nc.sync.dma_start(out=xt_all[:, 2, 1], in_=xt8_d[:, 2, 1])
        nc.sync.dma_start(out=xn_all[:, 1, 0], in_=xn8_d[:, 1, 0])
        nc.sync.dma_start(out=xn_all[:, 1, 1], in_=xn8_d[:, 1, 1])
        nc.sync.dma_start(out=xt_all[:, 3, 0], in_=xt8_d[:, 3, 0])
        nc.sync.dma_start(out=xt_all[:, 3, 1], in_=xt8_d[:, 3, 1])
        nc.sync.dma_start(out=xn_all[:, 2, 0], in_=xn8_d[:, 2, 0])
        nc.sync.dma_start(out=xn_all[:, 2, 1], in_=xn8_d[:, 2, 1])
        # xn3 ships as quarter-batch (128KB) descriptors: the tail waits
        # on the LAST arrival, and halving the final transfer halves the
        # v-MULT work still pending when the wire finishes
        nc.sync.dma_start(out=xn_all[:, 3, 0, 0:2], in_=xn8_d[:, 3, 0, 0:2])
        nc.sync.dma_start(out=xn_all[:, 3, 0, 2:4], in_=xn8_d[:, 3, 0, 2:4])
        nc.sync.dma_start(out=xn_all[:, 3, 1, 0:2], in_=xn8_d[:, 3, 1, 0:2])
        nc.sync.dma_start(out=xn_all[:, 3, 1, 2:4], in_=xn8_d[:, 3, 1, 2:4])

        # ---- scalar queue: tiny consts, then b0's xt halves (the
        # scalar preamble retires ~1us before sync's, so these bytes are
        # the first to land and drain in parallel with the sync queue);
        # ct2 (256KB, needed only by the pair tails) goes last ----
        wkb = const.tile([128, 2, 2, K], FP8E4)
        nc.scalar.dma_start(out=wkb[:], in_=wkb_d[:])
        b2 = const.tile([128, 1], F32)
        nc.scalar.dma_start(out=b2[:], in_=b2_d[:])
        ct2 = const.tile([128, D], F32)
        nc.scalar.dma_start(out=ct2[:], in_=ct2_d[:])

        # ---- PE warmup: ~3.4us of cold-clock matmuls releases the HAM
        # clock gate right as xt[0] lands. garb is memset first on the
        # DVE so the LDW has a ready operand without waiting on DMAs.
        garb = const.tile([128, 512], BF16)
        nc.vector.memset(garb[:], 0.25)
        # warm shares ps_v's "v" tag rotation (bank reused by pair1's v_ps)
        warm = ps_v.tile([128, 512], F32, tag="v", name="warm")
        for _ in range(N_WARM):
            nc.tensor.matmul(
                warm[:], garb[:, 0:128], garb[:], start=True, stop=True,
                skip_group_check=True,
            )

        # ---- small constants; identity built on-chip ----
        ones = const.tile([128, 1], BF16)
        nc.vector.memset(ones[:], 1.0)
        eps64 = const.tile([128, 1], F32)
        nc.vector.memset(eps64[:], float(64 * EPS))
        S_all = const.tile([128, 2], F32)
        it = const.tile([128, 65], F32)
        nc.gpsimd.iota(
            it[:], pattern=[[1, 65]], base=0, channel_multiplier=0,
            allow_small_or_imprecise_dtypes=True,
        )
        pidx = const.tile([128, 1], F32)
        nc.gpsimd.iota(
            pidx[:], pattern=[[0, 1]], base=0, channel_multiplier=1,
            allow_small_or_imprecise_dtypes=True,
        )
        pidx2 = const.tile([128, 1], F32)
        nc.gpsimd.iota(
            pidx2[:], pattern=[[0, 1]], base=-64, channel_multiplier=1,
            allow_small_or_imprecise_dtypes=True,
        )
        id_aug = const.tile([128, 65], BF16)
        nc.vector.tensor_scalar(
            id_aug[0:64, :], it[0:64, :], pidx[0:64, :], None, mybir.AluOpType.is_equal
        )
        nc.vector.tensor_scalar(
            id_aug[64:128, :], it[64:128, :], pidx2[64:128, :], None,
            mybir.AluOpType.is_equal,
        )
        nc.vector.memset(id_aug[:, 64:65], 1.0)

        v2 = {}
        as2 = {}
        vcs = {}

        def phase1(b):
            """mm1 + softmax -> a_sb[b]  (PE: 8x512 MULTs + 8 e-transposes).
            Each n-half gets its own [64,512] s_ps bank and eT tile at
            partitions 0:64 -- no cross-half WAR on a shared tile, so the
            scheduler never hoists a DMA-gated later batch ahead of the
            ready second half (which stalled the PE into a HAM re-throttle)."""
            invz = sb.tile([128, 8], F32, tag="invz")
            a_sb = sb.tile([128, 8, K], BF16, tag=f"a{b}")
            for h in range(2):
                s_ps = ps_s.tile([64, 512], F32, tag="s", name=f"s{b}h{h}", bufs=3)
                eT = sb.tile([64, 512], BF16, tag="eT", name=f"eT{b}h{h}", bufs=3)
                e_ps = ps_e.tile([128, 4, 65], F32, tag="e", bufs=2)
                # DoubleRow fp8: 2 packed weights/cell, 256-deep
                # contraction per MULT -- 2 MULTs per half instead of 4.
                # Wk ships pre-scaled by 64 (e4m3 subnormal floor); the
                # 1/64 folds into the Exp scale.
                for j in range(2):
                    nc.tensor.matmul(
                        s_ps[:],
                        wkb[:, j],
                        xt_all[:, b, h, j],
                        start=(j == 0),
                        stop=(j == 1),
                        perf_mode=mybir.MatmulPerfMode.DoubleRow,
                        skip_group_check=True,
                    )
                nc.scalar.activation(
                    eT[:], s_ps[:], ACTF.Exp, bias=b2[0:64, :], scale=0.015625
                )
                # out = eT_chunk^T @ [I64 | 1]: e natural + row-sums Z in col 64
                for c in range(4):
                    nc.tensor.matmul(
                        e_ps[:, c, :],
                        eT[:, c * 128 : (c + 1) * 128],
                        id_aug[0:64, :],
                        start=True,
                        stop=True,
                        skip_group_check=True,
                    )
                hs = slice(4 * h, 4 * (h + 1))
                nc.vector.reciprocal(invz[:, hs], e_ps[:, :, 64])
                nc.vector.tensor_mul(
                    a_sb[:, hs, :],
                    e_ps[:, :, 0:K],
                    invz[:, hs].broadcast_to([128, 4, K]),
                )
            return a_sb

        def phase2(b, a_sb):
            """mm2 + asum into pair-packed PSUM rows. For the last batch the
            asum group runs FIRST so the pair tail's vv (which needs both
            v_ps and as_ps) is gated only by the final v-MULT."""
            p2, h2 = b // 2, b % 2
            if h2 == 0:
                v2[p2] = ps_v.tile([128, 512], F32, tag="v", name=f"vps{p2}")
                as2[p2] = ps_as.tile([128, 1], F32, tag="as", name=f"as{p2}")
            v_ps = v2[p2]
            as_ps = as2[p2]
            rows = slice(64 * h2, 64 * (h2 + 1))

            def v_group():
                # all 8 v-MULTs back-to-back: weight double-buffer stays hot
                for c8 in range(8):
                    nc.tensor.matmul(
                        v_ps[rows, :],
                        a_sb[:, c8, :],
                        xn_all[:, b, c8 // 4, c8 % 4, :],
                        start=(c8 == 0),
                        stop=(c8 == 7),
                        skip_group_check=True,
                    )

            def as_group():
                # grouped 1-col asum MULTs (re-LDW is cheap; interleaving isn't)
                for c8 in range(8):
                    nc.tensor.matmul(
                        as_ps[rows, :],
                        a_sb[:, c8, :],
                        ones[:],
                        start=(c8 == 0),
                        stop=(c8 == 7),
                        skip_group_check=True,
                    )

            if b == 3:
                as_group()
                # pair1's ct2*asum product precomputes on the DVE while
                # the v-MULTs still run: only the cheap add remains on
                # the post-PE critical path
                vc1 = nrm.tile([128, D], F32, tag="vc1", name="vc1")
                nc.vector.tensor_scalar_mul(vc1[:], ct2[:], as_ps[:, 0:1])
                vcs[1] = vc1
                v_group()
            else:
                v_group()
                as_group()

        def tail_norm(p2):
            """v = asum*C^T + v_raw (fused stt); S = sum_d v^2 via accum_out;
            q = 8*sqrt(S+eps) on ACT (single Exp->Sqrt table switch);
            vfb = v/q. Pins: only SQRT(p0) (just after the model's last
            Exp, so the Sqrt table loads once) and pair1's whole chain
            (later still, so pair0's post-sqrt ops place first on the
            DVE and its store issues early)."""
            v_ps = v2[p2]
            A = mybir.AluOpType
            # bf16 intermediates: 2x DVE rate on the serial pair1 chain;
            # S still accumulates in f32 via accum_out
            vv = nrm.tile([128, D], BF16, tag=f"vv{p2}", name=f"vv{p2}")
            vsq = nrm.tile([128, D], BF16, tag=f"vsq{p2}", name=f"vsq{p2}")
            use_vc = p2 == 1
            q = nrm.tile([128, 1], F32, tag=f"q{p2}", name=f"q{p2}")
            sc = nrm.tile([128, 1], F32, tag=f"sc{p2}", name=f"sc{p2}")
            vfb = nrm.tile([128, D], BF16, tag=f"vfb{p2}", name=f"vfb{p2}")
            with tc.tile_wait_until(0.028, enable=(p2 == 1)):
                if use_vc:
                    nc.vector.tensor_add(vv[:], vcs[1][:], v_ps[:])
                else:
                    nc.vector.scalar_tensor_tensor(
                        vv[:], ct2[:], as2[p2][:, 0:1], v_ps[:], A.mult, A.add
                    )
                nc.vector.scalar_tensor_tensor(
                    vsq[:], vv[:], 1.0, vv[:], A.bypass, A.mult,
                    accum_out=S_all[:, p2 : p2 + 1],
                )
                with tc.tile_wait_until(0.024, enable=(p2 == 0)):
                    nc.scalar.activation(
                        q[:], S_all[:, p2 : p2 + 1], ACTF.Sqrt,
                        bias=eps64[:], scale=64.0,
                    )
                nc.vector.reciprocal(sc[:], q[:])
                if p2 == 1:
                    # final scale+store in column quarters, each with its
                    # own DRAM tensor (no WAW chain) striped over three
                    # idle DMA queues -- the 32KB stores drain in parallel
                    # and overlap the later quarters' scales
                    engs = [nc.gpsimd, nc.sync, nc.scalar, nc.gpsimd]
                    for u in range(4):
                        cs = slice(128 * u, 128 * (u + 1))
                        nc.vector.tensor_scalar_mul(vfb[:, cs], vv[:, cs], sc[:, 0:1])
                        engs[u].dma_start(out=out1q[u][:], in_=vfb[:, cs])
                else:
                    nc.vector.tensor_scalar_mul(vfb[:], vv[:], sc[:, 0:1])
            return vfb

        def tail_store(p2, vfb):
            """Direct k-major store (1KB runs); host does the tiny (64,512)
            per-batch transpose. Pair0 goes out on the scalar engine's DMA
            queue so it never waits behind the x input stream; pair1
            stores inline in tail_norm (split halves, two queues)."""
            nc.scalar.dma_start(out=out0_d[:], in_=vfb[:])

        # ---- interleaved schedule: no PE head-of-line blocking ----
        a0 = phase1(0)
        a1 = phase1(1)
        phase2(0, a0)
        a2 = phase1(2)
        phase2(1, a1)
        a3 = phase1(3)
        vfb0 = tail_norm(0)  # DVE-only; fills DVE idle gaps
        tail_store(0, vfb0)
        phase2(2, a2)
        phase2(3, a3)
        tail_norm(1)  # pair1 stores inside (split halves)

    nc.compile()
    return nc


_CACHED_NC = None


def _get_nc():
    global _CACHED_NC
    if _CACHED_NC is None:
        _CACHED_NC = build_kernel()
    return _CACHED_NC


def build_in_maps(x, Wk, b, C):
    import ml_dtypes

    B = x.shape[0]
    x2 = np.ascontiguousarray(x, dtype=np.float32).reshape(B, N, D)
    bpc = B // N_CORES
    Wkf = np.asarray(Wk, dtype=np.float32)
    Cf = np.asarray(C, dtype=np.float32)
    bf = np.asarray(b, dtype=np.float32).reshape(K)
    consts = {
        # d = j*256 + o*128 + p; DoubleRow packs (o) pairs per PE cell.
        # x64 lifts the 0.02-scale weights off the e4m3 subnormal floor.
        "wkb": np.ascontiguousarray(
            (Wkf * 64.0).reshape(2, 2, 128, K).transpose(2, 0, 1, 3)
        ).astype(ml_dtypes.float8_e4m3),
        "ct2": np.ascontiguousarray(np.concatenate([Cf.T, Cf.T], axis=0)),
        "b2": np.concatenate([bf, bf]).reshape(128, 1),
    }
    in_maps = []
    for c in range(N_CORES):
        A = x2[c * bpc : (c + 1) * bpc]  # (4, 1024, 512)
        # xn8[p, b, h, c, d]: pixel n = (4h+c)*128 + p
        xn8 = np.ascontiguousarray(
            A.reshape(bpc, 2, 4, 128, D).transpose(3, 0, 1, 2, 4)
        ).astype(ml_dtypes.float8_e3m4)
        # xt8[p, b, h, j, o, nn]: d = j*256 + o*128 + p, n = h*512 + nn
        xt8 = np.ascontiguousarray(
            A.transpose(0, 2, 1)
            .reshape(bpc, 2, 2, 128, 2, 512)
            .transpose(3, 0, 4, 1, 2, 5)
        ).astype(ml_dtypes.float8_e4m3)
        in_maps.append({"xn8": xn8, "xt8": xt8, **consts})
    return in_maps


def kernel(x, Wk, b, C):
    """Full-input NetVLAD forward. x (32,32,32,512) f32 -> out (32, 32768) f32."""
    in_maps = build_in_maps(x, Wk, b, C)
    nc = _get_nc()
    res = run_bass_kernel_spmd(nc, in_maps, list(range(N_CORES)))
    outs = []
    for c in range(N_CORES):
        o = np.stack(
            [
                np.asarray(res.results[c]["out0"]),
                np.concatenate(
                    [np.asarray(res.results[c][f"out1{u}"]) for u in range(4)],
                    axis=1,
                ),
            ],
            axis=1,
        )  # (128, 2, 512) bf16: p=64*hh+k
        outs.append(
            o.reshape(2, K, 2, D)
            .transpose(2, 0, 3, 1)
            .reshape(B_PER_CORE, D * K)
            .astype(np.float32)
        )
    return np.concatenate(outs, axis=0)
